# revision 11
# baseline (speedup 1.0000x reference)
"""ChannelSA Trainium2 kernel: 8-way batch-parallel across NeuronCores.

kernel(**inputs) takes the FULL inputs (x [8,192,128,128], conv weights,
pos_emb) and returns the FULL output [8,192,128,128] fp32. Each core runs
an identical single-batch program (SPMD, no collectives).

Per-core pipeline (v2: fp8 DoubleRow on the q,k path):
  q,k path is scale-invariant (logits are the normalized Gram
  Gqk/(|q||k|), softmax is shift-invariant) so it runs entirely in
  fp8e4m3 with prescaled weights at the PE's DoubleRow rate (2 K-tiles
  per pass, 0.5 cyc/col):
    zqk = W1qk @ x        one DR matmul per 128-out block (K=192 in 2 k-tiles)
    q,k = DW3x3(zqk)      9 taps as 5 DR diag-pair matmuls per block
  v path stays bf16 for accuracy:
    zv  = W1v @ x         K=128+64 accumulating matmuls
    v   = DW3x3(zv)       block3: 9 diag matmuls; block4 (64ch): z stored
                          twice (partitions 64:128 col-shifted) so taps
                          (di,-1)+(di,0) share one matmul -> 6 matmuls
  q,k -> bf16 DMA-transpose -> per-head Gram banks [Gqk|Gqq|Gkk] in PSUM
  logits = Gqk / (|q||k| sqrt(L))  (pos_emb is a per-row constant: no-op)
  attn = softmax(logits)
  y = (W_out @ blockdiag(attn)) @ v
"""
import math
from contextlib import ExitStack

import numpy as np

import concourse.ap as cap
import concourse.bass as bass
import concourse.mybir as mybir
import concourse.tile as tile
from concourse.masks import make_identity

F32 = mybir.dt.float32
BF16 = mybir.dt.bfloat16
F8 = mybir.dt.float8e4
AF = mybir.ActivationFunctionType
DR = mybir.MatmulPerfMode.DoubleRow

C = 192
CQKV = 576
H = 128
W = 128
L = H * W
HEADS = 4
DH = 48
R = 8                    # output image rows per chunk
NCHUNK = H // R
PADW = W + 2             # padded row stride in z tiles
ZROWS = R + 2            # rows held per z chunk (1 halo each side)
TAPS = [(di, dj) for di in (-1, 0, 1) for dj in (-1, 0, 1)]
N_CORES = 8
BLK4_PAIR = False        # block4 dw tap-pair trick (partition dup)
FP8_DW = False           # fp8 DoubleRow dw for q,k blocks
S1 = 16.0                # fp8 prescale on W1 qk columns
S2 = 4.0                 # fp8 prescale on dw qk weights

_MAX_DRAIN_WAITS = 1


def _patch_tail_drain():
    """The walrus in this image rejects >1 semaphore wait on the Tile tail
    drain instruction; split the waits across a chain of SP nops."""
    if getattr(tile.TileContext, "_drain_patched", False):
        return

    def _drain_and_barrier(self, tick_clock, wait_clock):
        from concourse.vector_clock import ScopedClock

        nc = self.nc
        drain_inst = nc.sync.drain()
        wait_clock.add_sem_waits(
            drain_inst.ins, ScopedClock({None: tick_clock.global_clock})
        )
        si = drain_inst.ins.sync_info
        waits = list(si.on_wait or [])
        if len(waits) > _MAX_DRAIN_WAITS:
            si.on_wait = waits[:_MAX_DRAIN_WAITS]
            rest = waits[_MAX_DRAIN_WAITS:]
            for i in range(0, len(rest), _MAX_DRAIN_WAITS):
                nop = nc.sync.nop(nofuse=True)
                nop.ins.sync_info = mybir.SyncInfo(
                    on_wait=rest[i : i + _MAX_DRAIN_WAITS], on_update=[]
                )
        nc.all_engine_barrier()
        assert self.sems is not None
        popped = nc._tile_sem_poison_stack.pop()
        assert popped is self._sem_poison
        nc.clear_and_free_semaphores(list(self.sems.allocated().values()))
        nc.all_engine_barrier()

    tile.TileContext._drain_and_barrier = _drain_and_barrier
    tile.TileContext._drain_patched = True


def build_nc(split_waits=True):
    _patch_tail_drain()
    nc = bass.Bass("TRN2", target_bir_lowering=False, debug=False)

    x_d = nc.declare_dram_parameter("x", [C, L], BF16, isOutput=False)
    xf8_d = nc.declare_dram_parameter("xf8", [C, L], F8, isOutput=False)
    w1qk_d = nc.declare_dram_parameter("w1qk", [128, 2 * 384], F8, isOutput=False)
    w1tv_d = nc.declare_dram_parameter("w1tv", [C, C], BF16, isOutput=False)
    wqk8_d = nc.declare_dram_parameter("wqk8", [128, 30 * 128], F8, isOutput=False)
    w3_d = nc.declare_dram_parameter("w3", [128, 9 * 128], BF16, isOutput=False)
    w4_d = nc.declare_dram_parameter("w4", [128, 6 * 64], BF16, isOutput=False)
    w4s_d = nc.declare_dram_parameter("w4s", [64, 9 * 64], BF16, isOutput=False)
    wqs_d = nc.declare_dram_parameter("wqs", [128, 27 * 128], BF16, isOutput=False)
    woutt_d = nc.declare_dram_parameter("woutt", [C, C], F32, isOutput=False)
    y_d = nc.declare_dram_parameter("y", [C, L], F32, isOutput=True)

    with tile.TileContext(nc) as tc, ExitStack() as ctx:
        _body(ctx, tc, x_d, xf8_d, w1qk_d, w1tv_d, wqk8_d, w3_d, w4_d, w4s_d, wqs_d, woutt_d, y_d)
    if split_waits:
        _split_excess_waits(nc)
    return nc


def _split_excess_waits(nc, maxw=1):
    """This walrus build accepts only one semaphore wait per instruction.
    Move excess waits onto same-engine no-ops inserted just before the
    offending instruction (same-engine program order preserves semantics)."""
    uid = [0]
    for f in nc.m.functions:
        for bb in f.blocks:
            il = bb.instructions
            out = []
            changed = False
            for inst in il:
                si = inst.sync_info
                waits = list(si.on_wait) if si and si.on_wait else []
                if len(waits) > maxw:
                    changed = True
                    rest, keep = waits[:-maxw], waits[-maxw:]
                    for i in range(0, len(rest), maxw):
                        uid[0] += 1
                        out.append(
                            mybir.InstNoOp(
                                name=f"I-waitsplit-{uid[0]}",
                                engine=inst.engine,
                                ins=[],
                                outs=[],
                                sync_info=mybir.SyncInfo(
                                    on_wait=rest[i : i + maxw], on_update=[]
                                ),
                            )
                        )
                    si.on_wait = keep
                out.append(inst)
            if changed:
                bb.instructions = out


def _tap_pair_ap(ztile, orow, p):
    """rhs AP [128, 2(k-tiles), 4(rows), W] for dw tap pair p over a
    [128, ZROWS, PADW] z tile; k-tile t reads tap TAPS[2p+t]."""
    t0 = TAPS[2 * p]
    off0 = (orow + t0[0]) * PADW + (1 + t0[1])
    if 2 * p + 1 < 9:
        t1 = TAPS[2 * p + 1]
        delta = (orow + t1[0]) * PADW + (1 + t1[1]) - off0
    else:
        delta = 0  # pad pair: zero weights, reread tap t0 (stride-0 is legal)
    base = ztile[:]
    pstride = base.ap[0][0]
    return cap.AP(
        base.tensor,
        base.offset + off0,
        [[pstride, 128], [delta, 2], [PADW, 4], [1, W]],
    )


def _body(ctx, tc, x_d, xf8_d, w1qk_d, w1tv_d, wqk8_d, w3_d, w4_d, w4s_d, wqs_d, woutt_d, y_d):
    nc = tc.nc
    ncopy = [0]

    def copy(dst, src):
        # alternate PSUM->SBUF copies between ACT and DVE
        if ncopy[0] % 2 == 0:
            nc.scalar.copy(dst, src)
        else:
            nc.vector.tensor_copy(dst, src)
        ncopy[0] += 1

    const = ctx.enter_context(tc.tile_pool(name="const", bufs=1))
    persist = ctx.enter_context(tc.tile_pool(name="persist", bufs=1))

    # ---- constants / weights ----
    w1qk = const.tile([128, 2, 384], F8, tag="w1qk")
    nc.sync.dma_start(w1qk[:], w1qk_d[:].rearrange("k (t m) -> k t m", t=2))
    wqk8 = const.tile([128, 30, 128], F8, tag="wqk8")
    nc.sync.dma_start(wqk8[:], wqk8_d[:].rearrange("k (p m) -> k p m", p=30))
    w3 = const.tile([128, 9, 128], BF16, tag="w3")
    nc.sync.dma_start(w3[:], w3_d[:].rearrange("k (t m) -> k t m", t=9))
    w4 = const.tile([128, 6, 64], BF16, tag="w4")
    nc.sync.dma_start(w4[:], w4_d[:].rearrange("k (t m) -> k t m", t=6))
    w4s = const.tile([64, 9, 64], BF16, tag="w4s")
    nc.sync.dma_start(w4s[:], w4s_d[:].rearrange("k (t m) -> k t m", t=9))
    if not FP8_DW:
        wqs = const.tile([128, 27, 128], BF16, tag="wqs")
        nc.sync.dma_start(wqs[:], wqs_d[:].rearrange("k (t m) -> k t m", t=27))

    w1tv0 = const.tile([128, C], BF16, tag="w1tv0")
    w1tv1 = const.tile([128, C], BF16, tag="w1tv1")
    nc.gpsimd.memset(w1tv1[:], 0.0)
    nc.sync.dma_start(w1tv0[:], w1tv_d[0:128, :])
    nc.sync.dma_start(w1tv1[0:64, :], w1tv_d[128:192, :])

    woutt0 = const.tile([128, C], F32, tag="woutt0")
    woutt1 = const.tile([64, C], F32, tag="woutt1")
    nc.sync.dma_start(woutt0[:], woutt_d[0:128, :])
    nc.sync.dma_start(woutt1[:], woutt_d[128:192, :])
    woutt0_bf = const.tile([128, C], BF16, tag="woutt0bf")
    woutt1_bf = const.tile([64, C], BF16, tag="woutt1bf")
    nc.vector.tensor_copy(woutt0_bf[:], woutt0[:])
    nc.vector.tensor_copy(woutt1_bf[:], woutt1[:])

    ident48 = const.tile([48, 48], F32, tag="ident48")
    make_identity(nc, ident48[:])
    ones48 = const.tile([48, 1], F32, tag="ones48")
    nc.gpsimd.memset(ones48[:], 1.0)
    ones1x48 = const.tile([1, 48], F32, tag="ones1x48")
    nc.gpsimd.memset(ones1x48[:], 1.0)

    # ---- persistent state ----
    v0 = persist.tile([128, L], BF16, tag="v0")
    v1 = persist.tile([128, L], BF16, tag="v1")
    nc.gpsimd.memset(v1[64:128, :], 0.0)
    ZQDT = F8 if FP8_DW else BF16
    zqk = [
        [
            persist.tile([128, ZROWS, PADW], ZQDT, tag=f"zqk{s}_{b}", name=f"zqk{s}_{b}")
            for b in range(3)
        ]
        for s in range(2)
    ]
    zv3 = [persist.tile([128, ZROWS, PADW], BF16, tag=f"zv3_{s}", name=f"zv3_{s}") for s in range(2)]
    zv4 = [persist.tile([128, ZROWS, PADW], BF16, tag=f"zv4_{s}", name=f"zv4_{s}") for s in range(2)]
    for s in range(2):
        for b in range(3):
            nc.gpsimd.memset(zqk[s][b][:], 0.0)
        nc.gpsimd.memset(zv3[s][:], 0.0)
        nc.gpsimd.memset(zv4[s][:], 0.0)

    ghs = persist.tile([48, HEADS * 144], F32, tag="ghs")
    xt1_pp = [persist.tile([128, ZROWS, W], BF16, tag=f"xt1_{s}", name=f"xt1_{s}") for s in range(2)]
    xf8_pp = [
        persist.tile([128, 2, ZROWS, W], F8, tag=f"xf8_{s}", name=f"xf8_{s}") for s in range(2)
    ]
    for s in range(2):
        nc.gpsimd.memset(xt1_pp[s][:], 0.0)
        nc.gpsimd.memset(xf8_pp[s][:], 0.0)

    # ---- phase A: chunked pipeline ----
    with (
        tc.tile_pool(name="gps", bufs=1, space="PSUM") as gps,
        tc.tile_pool(name="xp", bufs=2) as xp,
        tc.tile_pool(name="zps", bufs=3, space="PSUM") as zps,
        tc.tile_pool(name="qps", bufs=2, space="PSUM") as qps,
        tc.tile_pool(name="stp", bufs=2) as stp,
        tc.tile_pool(name="qktp", bufs=2) as qktp,
    ):
        # two G banks; a single accumulation group spans all heads per bank
        g1 = gps.tile([48, HEADS * 96], F32, tag="g1")
        g2 = gps.tile([48, HEADS * 48], F32, tag="g2")
        for c in range(NCHUNK):
            zq = zqk[c % 2]
            z3 = zv3[c % 2]
            z4 = zv4[c % 2]
            r0 = max(0, R * c - 1)
            r1 = min(H, R * c + R + 1)
            nrows = r1 - r0
            brow0 = r0 - (R * c - 1)  # buf row of image row r0

            xt0 = xp.tile([128, nrows, W], BF16, tag="x0")
            xt1 = xt1_pp[c % 2]
            xf8 = xf8_pp[c % 2]
            nc.sync.dma_start(
                xt0[:], x_d[0:128, r0 * W : r1 * W].rearrange("p (r w) -> p r w", w=W)
            )
            nc.sync.dma_start(
                xt1[0:64, 0:nrows, :],
                x_d[128:192, r0 * W : r1 * W].rearrange("p (r w) -> p r w", w=W),
            )
            nc.sync.dma_start(
                xf8[:, 0, 0:nrows, :],
                xf8_d[0:128, r0 * W : r1 * W].rearrange("p (r w) -> p r w", w=W),
            )
            nc.sync.dma_start(
                xf8[0:64, 1, 0:nrows, :],
                xf8_d[128:192, r0 * W : r1 * W].rearrange("p (r w) -> p r w", w=W),
            )

            # conv1 into padded z tiles (groups of <=4 rows)
            for g0 in range(0, nrows, 4):
                gn = min(4, nrows - g0)
                rsl = slice(brow0 + g0, brow0 + g0 + gn)
                for b in range(3):
                    ps = zps.tile([128, 512], F32, tag="zps")
                    nc.tensor.matmul(
                        ps[:, 0 : gn * W],
                        w1qk[:, :, b * 128 : (b + 1) * 128],
                        xf8[:, :, g0 : g0 + gn, :],
                        start=True,
                        stop=True,
                        perf_mode=DR,
                    )
                    copy(zq[b][:, rsl, 1 : 1 + W], ps[:, 0 : gn * W])
                ps3 = zps.tile([128, 512], F32, tag="zps")
                nc.tensor.matmul(
                    ps3[:, 0 : gn * W], w1tv0[:, 0:128], xt0[:, g0 : g0 + gn, :],
                    start=True, stop=False,
                )
                nc.tensor.matmul(
                    ps3[:, 0 : gn * W], w1tv1[:, 0:128], xt1[:, g0 : g0 + gn, :],
                    start=False, stop=True,
                )
                copy(z3[:, rsl, 1 : 1 + W], ps3[:, 0 : gn * W])
                ps4 = zps.tile([128, 512], F32, tag="zps")
                nc.tensor.matmul(
                    ps4[0:64, 0 : gn * W], w1tv0[:, 128:192], xt0[:, g0 : g0 + gn, :],
                    start=True, stop=False,
                )
                nc.tensor.matmul(
                    ps4[0:64, 0 : gn * W], w1tv1[:, 128:192], xt1[:, g0 : g0 + gn, :],
                    start=False, stop=True,
                )
                copy(z4[0:64, rsl, 1 : 1 + W], ps4[0:64, 0 : gn * W])
                if BLK4_PAIR:
                    # partition-shifted duplicate (col -1) for the dw tap-pair
                    # trick; engines can't cross partitions, so DMA the bf16 copy
                    nc.scalar.dma_start(z4[64:128, rsl, 0:W], z4[0:64, rsl, 1 : 1 + W])

            if c == NCHUNK - 1:
                # bottom halo row never written this chunk; clear stale data
                for b in range(3):
                    nc.gpsimd.memset(zq[b][:, ZROWS - 1 : ZROWS, :], 0.0)
                nc.gpsimd.memset(z3[:, ZROWS - 1 : ZROWS, :], 0.0)
                nc.gpsimd.memset(z4[:, ZROWS - 1 : ZROWS, :], 0.0)

            # dw taps -> qkv rows Rc..Rc+R
            st = [stp.tile([128, R // 4, 4 * W], BF16, tag=f"st{i}", name=f"st{i}") for i in range(3)]
            for g in range(R // 4):
                orow = 1 + 4 * g  # buf row of first output row in this group
                for b in range(3):
                    ps = qps.tile([128, 512], F32, tag="qps")
                    if FP8_DW:
                        for p in range(5):
                            nc.tensor.matmul(
                                ps[:, :],
                                wqk8[:, (b * 5 + p) * 2 : (b * 5 + p) * 2 + 2, :],
                                _tap_pair_ap(zq[b], orow, p),
                                start=(p == 0),
                                stop=(p == 4),
                                perf_mode=DR,
                            )
                    else:
                        for t, (di, dj) in enumerate(TAPS):
                            nc.tensor.matmul(
                                ps[:, :],
                                wqs[:, b * 9 + t, :],
                                zq[b][:, orow + di : orow + di + 4, 1 + dj : 1 + dj + W],
                                start=(t == 0),
                                stop=(t == 8),
                            )
                    copy(st[b][:, g, :], ps[:, :])
                ps3 = qps.tile([128, 512], F32, tag="qps")
                for t, (di, dj) in enumerate(TAPS):
                    nc.tensor.matmul(
                        ps3[:, :],
                        w3[:, t, :],
                        z3[:, orow + di : orow + di + 4, 1 + dj : 1 + dj + W],
                        start=(t == 0),
                        stop=(t == 8),
                    )
                copy(v0[:, c * R * W + g * 512 : c * R * W + (g + 1) * 512], ps3[:, :])
                ps4 = qps.tile([64, 512], F32, tag="qps4", bufs=1)
                if BLK4_PAIR:
                    for i, di in enumerate((-1, 0, 1)):
                        nc.tensor.matmul(
                            ps4[:, :],
                            w4[:, 2 * i, :],
                            z4[:, orow + di : orow + di + 4, 0:W],
                            start=(i == 0),
                            stop=False,
                        )
                        nc.tensor.matmul(
                            ps4[:, :],
                            w4[0:64, 2 * i + 1, :],
                            z4[0:64, orow + di : orow + di + 4, 2 : 2 + W],
                            start=False,
                            stop=(i == 2),
                        )
                else:
                    for t, (di, dj) in enumerate(TAPS):
                        nc.tensor.matmul(
                            ps4[:, :],
                            w4s[0:64, t, :],
                            z4[0:64, orow + di : orow + di + 4, 1 + dj : 1 + dj + W],
                            start=(t == 0),
                            stop=(t == 8),
                        )
                copy(v1[0:64, c * R * W + g * 512 : c * R * W + (g + 1) * 512], ps4[:, :])

            # transpose q,k: qkt[:, lt, 0, :] = k^T, [:, lt, 1, :] = q^T
            st_flat = [s.rearrange("p a b -> p (a b)") for s in st]
            qkt = qktp.tile([128, R, 2, 192], BF16, tag="qkt")
            nc.sync.dma_start_transpose(qkt[:, :, 1, 0:128], st_flat[0][:, :])
            nc.scalar.dma_start_transpose(qkt[:, :, 1, 128:192], st_flat[1][0:64, :])
            nc.sync.dma_start_transpose(qkt[:, :, 0, 0:64], st_flat[1][64:128, :])
            nc.scalar.dma_start_transpose(qkt[:, :, 0, 64:192], st_flat[2][:, :])

            # gram accumulation
            for lt in range(R):
                first = c == 0 and lt == 0
                last = c == NCHUNK - 1 and lt == R - 1
                for h in range(HEADS):
                    nc.tensor.matmul(
                        g1[:, h * 96 : h * 96 + 96],
                        qkt[:, lt, 1, h * DH : (h + 1) * DH],
                        qkt[:, lt, :, h * DH : (h + 1) * DH],
                        start=(first and h == 0),
                        stop=(last and h == HEADS - 1),
                        skip_group_check=True,
                    )
                    nc.tensor.matmul(
                        g2[:, h * DH : (h + 1) * DH],
                        qkt[:, lt, 0, h * DH : (h + 1) * DH],
                        qkt[:, lt, 0, h * DH : (h + 1) * DH],
                        start=(first and h == 0),
                        stop=(last and h == HEADS - 1),
                        skip_group_check=True,
                    )

        nc.vector.tensor_copy(ghs[:, 0 : HEADS * 96], g1[:])
        nc.vector.tensor_copy(ghs[:, HEADS * 96 :], g2[:])

    # ---- phase B ----
    with (
        tc.tile_pool(name="bsb", bufs=1) as bsb,
        tc.tile_pool(name="bps", bufs=1, space="PSUM") as bps,
        tc.tile_pool(name="ops", bufs=4, space="PSUM") as ops,
        tc.tile_pool(name="osb", bufs=4) as osb,
    ):
        attn_bf = bsb.tile([48, HEADS * 48], BF16, tag="attnbf")
        scr = bsb.tile([48, 48], F32, tag="scr")
        scr2 = bsb.tile([48, 48], F32, tag="scr2")
        colv = bsb.tile([48, 1], F32, tag="colv")
        rowv = bsb.tile([1, 48], F32, tag="rowv")
        rkrep = bsb.tile([48, 48], F32, tag="rkrep")
        logits = bsb.tile([48, 48], F32, tag="logits")

        for h in range(HEADS):
            gqk = ghs[:, h * 96 : h * 96 + 48]
            gqq = ghs[:, h * 96 + 48 : h * 96 + 96]
            gkk = ghs[:, HEADS * 96 + h * DH : HEADS * 96 + (h + 1) * DH]

            # rq_inv = 1/max(sqrt(diag(Gqq)),eps); 1/sqrt(L) scale is constant
            # across the softmax row only if folded for both q and k norms
            nc.vector.tensor_mul(scr[:], gqq, ident48[:])
            nc.vector.reduce_sum(colv[:], scr[:], axis=mybir.AxisListType.X)
            nc.scalar.activation(colv[:], colv[:], AF.Sqrt)
            nc.vector.tensor_scalar_max(colv[:], colv[:], 1e-12)
            nc.vector.reciprocal(colv[:], colv[:])
            nc.vector.tensor_scalar(
                logits[:],
                gqk,
                colv[:],
                1.0 / math.sqrt(L),
                op0=mybir.AluOpType.mult,
                op1=mybir.AluOpType.mult,
            )

            # rk_inv broadcast along the free (key) dim via diag-as-row
            nc.vector.tensor_mul(scr2[:], gkk, ident48[:])
            ps_row = bps.tile([1, 48], F32, tag="pssmall")
            nc.tensor.matmul(ps_row[:], ones48[:], scr2[:], start=True, stop=True)
            nc.vector.tensor_copy(rowv[:], ps_row[:])
            nc.scalar.activation(rowv[:], rowv[:], AF.Sqrt)
            nc.vector.tensor_scalar_max(rowv[:], rowv[:], 1e-12)
            nc.vector.reciprocal(rowv[:], rowv[:])
            ps_rep = bps.tile([48, 48], F32, tag="pssmall")
            nc.tensor.matmul(ps_rep[:], ones1x48[:], rowv[:], start=True, stop=True)
            nc.vector.tensor_copy(rkrep[:], ps_rep[:])
            nc.vector.tensor_mul(logits[:], logits[:], rkrep[:])

            # softmax over the free (key) dim
            nc.vector.reduce_max(colv[:], logits[:], axis=mybir.AxisListType.X)
            nc.vector.tensor_scalar_sub(logits[:], logits[:], colv[:])
            nc.scalar.activation(logits[:], logits[:], AF.Exp)
            nc.vector.reduce_sum(colv[:], logits[:], axis=mybir.AxisListType.X)
            nc.vector.reciprocal(colv[:], colv[:])
            nc.vector.tensor_scalar_mul(logits[:], logits[:], colv[:])
            nc.vector.tensor_copy(attn_bf[:, h * 48 : (h + 1) * 48], logits[:])

        # block-diagonal attn (bf16)
        bd0 = bsb.tile([128, C], BF16, tag="bd0")
        bd1 = bsb.tile([64, C], BF16, tag="bd1")
        nc.gpsimd.memset(bd0[:], 0.0)
        nc.gpsimd.memset(bd1[:], 0.0)
        nc.sync.dma_start(bd0[0:48, 0:48], attn_bf[:, 0:48])
        nc.sync.dma_start(bd0[48:96, 48:96], attn_bf[:, 48:96])
        nc.sync.dma_start(bd0[96:128, 96:144], attn_bf[0:32, 96:144])
        nc.sync.dma_start(bd1[0:16, 96:144], attn_bf[32:48, 96:144])
        nc.sync.dma_start(bd1[16:64, 144:192], attn_bf[:, 144:192])

        # W_effT = BD(attn).T @ W_outT   [192 x 192], bf16
        weff0 = bsb.tile([128, 256], BF16, tag="weff0")
        weff1 = bsb.tile([128, 256], BF16, tag="weff1")
        nc.gpsimd.memset(weff0[:], 0.0)
        nc.gpsimd.memset(weff1[:], 0.0)
        for m0, m1, wt in [(0, 128, weff0), (128, 192, weff1)]:
            pw = bps.tile([128, C], F32, tag="pweff")
            nc.tensor.matmul(pw[0 : m1 - m0, :], bd0[:, m0:m1], woutt0_bf[:], start=True, stop=False)
            nc.tensor.matmul(pw[0 : m1 - m0, :], bd1[:, m0:m1], woutt1_bf[:], start=False, stop=True)
            copy(wt[0 : m1 - m0, 0:C], pw[0 : m1 - m0, :])

        # y = W_effT.T @ v
        for g in range(L // 512):
            sl = slice(g * 512, (g + 1) * 512)
            for m0, m1 in [(0, 128), (128, 192)]:
                po = ops.tile([128, 512], F32, tag="ops")
                nc.tensor.matmul(po[:, :], weff0[:, m0 : m0 + 128], v0[:, sl], start=True, stop=False)
                nc.tensor.matmul(po[:, :], weff1[:, m0 : m0 + 128], v1[:, sl], start=False, stop=True)
                ot = osb.tile([m1 - m0, 512], F32, tag=f"o{m0}", name=f"o{m0}")
                copy(ot[:], po[0 : m1 - m0, :])
                nc.sync.dma_start(y_d[m0:m1, sl], ot[:])


def _tap_idx(di, dj):
    return 3 * (di + 1) + (dj + 1)


def _prep_weights(w_proj1, w_dw, w_out):
    import ml_dtypes

    E4 = ml_dtypes.float8_e4m3
    w1t = np.asarray(w_proj1, np.float32).reshape(CQKV, C).T  # [in, out]
    wdw = np.asarray(w_dw, np.float32).reshape(CQKV, 9)
    woutt = np.ascontiguousarray(np.asarray(w_out, np.float32).reshape(C, C).T)

    w1qk = np.zeros((128, 2, 384), np.float32)
    w1qk[:, 0, :] = w1t[0:128, 0:384] * S1
    w1qk[0:64, 1, :] = w1t[128:192, 0:384] * S1
    w1qk = np.ascontiguousarray(w1qk.reshape(128, 768)).astype(E4)

    w1tv = np.ascontiguousarray(w1t[:, 384:576]).astype(ml_dtypes.bfloat16)

    wqk8 = np.zeros((128, 30, 128), np.float32)
    for b in range(3):
        for p in range(5):
            for s in range(2):
                t = 2 * p + s
                if t < 9:
                    np.fill_diagonal(wqk8[:, (b * 5 + p) * 2 + s, :], wdw[b * 128 : (b + 1) * 128, t] * S2)
    wqk8 = np.ascontiguousarray(wqk8.reshape(128, 3840)).astype(E4)

    w3 = np.zeros((128, 9, 128), np.float32)
    for t in range(9):
        np.fill_diagonal(w3[:, t, :], wdw[384:512, t])
    w3 = np.ascontiguousarray(w3.reshape(128, 1152)).astype(ml_dtypes.bfloat16)

    w4 = np.zeros((128, 6, 64), np.float32)
    for i, di in enumerate((-1, 0, 1)):
        np.fill_diagonal(w4[0:64, 2 * i, :], wdw[512:576, _tap_idx(di, -1)])
        np.fill_diagonal(w4[64:128, 2 * i, :], wdw[512:576, _tap_idx(di, 0)])
        np.fill_diagonal(w4[0:64, 2 * i + 1, :], wdw[512:576, _tap_idx(di, 1)])
    w4 = np.ascontiguousarray(w4.reshape(128, 384)).astype(ml_dtypes.bfloat16)

    w4s = np.zeros((64, 9, 64), np.float32)
    for t in range(9):
        np.fill_diagonal(w4s[:, t, :], wdw[512:576, t])
    w4s = np.ascontiguousarray(w4s.reshape(64, 576)).astype(ml_dtypes.bfloat16)

    wqs = np.zeros((128, 27, 128), np.float32)
    for b in range(3):
        for t in range(9):
            np.fill_diagonal(wqs[:, b * 9 + t, :], wdw[b * 128 : (b + 1) * 128, t])
    wqs = np.ascontiguousarray(wqs.reshape(128, 27 * 128)).astype(ml_dtypes.bfloat16)

    return {
        "w1qk": w1qk,
        "w1tv": w1tv,
        "wqk8": wqk8,
        "w3": w3,
        "w4": w4,
        "w4s": w4s,
        "wqs": wqs,
        "woutt": woutt,
    }


_NC_CACHE = None


def _get_nc():
    global _NC_CACHE
    if _NC_CACHE is None:
        _NC_CACHE = build_nc()
    return _NC_CACHE


def kernel(x, w_proj1, w_dw, pos_emb, w_out, _trace=False):
    from concourse.bass_utils import run_bass_kernel_spmd

    import ml_dtypes

    xf = np.asarray(x, dtype=np.float32)
    xbf = xf.astype(ml_dtypes.bfloat16)
    xf8 = xf.astype(ml_dtypes.float8_e4m3)
    wmaps = _prep_weights(w_proj1, w_dw, w_out)
    # pos_emb adds a per-head constant to every logit in its softmax row;
    # softmax is shift-invariant, so it has no effect on the output.

    nc = _get_nc()
    in_maps = [
        {
            "x": np.ascontiguousarray(xbf[b].reshape(C, L)),
            "xf8": np.ascontiguousarray(xf8[b].reshape(C, L)),
            **wmaps,
        }
        for b in range(N_CORES)
    ]
    res = run_bass_kernel_spmd(nc, in_maps, list(range(N_CORES)), trace=_trace)
    out = np.stack([res.results[b]["y"].reshape(C, H, W) for b in range(N_CORES)])
    if _trace:
        kernel.last_exec_time_ns = res.exec_time_ns
        kernel.last_profile = res
    return out.astype(np.float32)


# revision 12
# speedup vs baseline: 1.0025x; 1.0025x over previous
"""ChannelSA Trainium2 kernel: 8-way batch-parallel across NeuronCores.

kernel(**inputs) takes the FULL inputs (x [8,192,128,128], conv weights,
pos_emb) and returns the FULL output [8,192,128,128] fp32. Each core runs
an identical single-batch program (SPMD, no collectives).

Per-core pipeline (v2: fp8 DoubleRow on the q,k path):
  q,k path is scale-invariant (logits are the normalized Gram
  Gqk/(|q||k|), softmax is shift-invariant) so it runs entirely in
  fp8e4m3 with prescaled weights at the PE's DoubleRow rate (2 K-tiles
  per pass, 0.5 cyc/col):
    zqk = W1qk @ x        one DR matmul per 128-out block (K=192 in 2 k-tiles)
    q,k = DW3x3(zqk)      9 taps as 5 DR diag-pair matmuls per block
  v path stays bf16 for accuracy:
    zv  = W1v @ x         K=128+64 accumulating matmuls
    v   = DW3x3(zv)       block3: 9 diag matmuls; block4 (64ch): z stored
                          twice (partitions 64:128 col-shifted) so taps
                          (di,-1)+(di,0) share one matmul -> 6 matmuls
  q,k -> bf16 DMA-transpose -> per-head Gram banks [Gqk|Gqq|Gkk] in PSUM
  logits = Gqk / (|q||k| sqrt(L))  (pos_emb is a per-row constant: no-op)
  attn = softmax(logits)
  y = (W_out @ blockdiag(attn)) @ v
"""
import math
from contextlib import ExitStack

import numpy as np

import concourse.ap as cap
import concourse.bass as bass
import concourse.mybir as mybir
import concourse.tile as tile
from concourse.masks import make_identity

F32 = mybir.dt.float32
BF16 = mybir.dt.bfloat16
F8 = mybir.dt.float8e4
AF = mybir.ActivationFunctionType
DR = mybir.MatmulPerfMode.DoubleRow

C = 192
CQKV = 576
H = 128
W = 128
L = H * W
HEADS = 4
DH = 48
R = 8                    # output image rows per chunk
NCHUNK = H // R
PADW = W + 2             # padded row stride in z tiles
ZROWS = R + 2            # rows held per z chunk (1 halo each side)
TAPS = [(di, dj) for di in (-1, 0, 1) for dj in (-1, 0, 1)]
N_CORES = 8
BLK4_PAIR = False        # block4 dw tap-pair trick (partition dup)
FP8_DW = 2               # q,k dw mode: 0=bf16 diag, 1=fp8 DoubleRow pairs, 2=fp8 singles
S1 = 16.0                # fp8 prescale on W1 qk columns
S2 = 4.0                 # fp8 prescale on dw qk weights

_MAX_DRAIN_WAITS = 1


def _patch_tail_drain():
    """The walrus in this image rejects >1 semaphore wait on the Tile tail
    drain instruction; split the waits across a chain of SP nops."""
    if getattr(tile.TileContext, "_drain_patched", False):
        return

    def _drain_and_barrier(self, tick_clock, wait_clock):
        from concourse.vector_clock import ScopedClock

        nc = self.nc
        drain_inst = nc.sync.drain()
        wait_clock.add_sem_waits(
            drain_inst.ins, ScopedClock({None: tick_clock.global_clock})
        )
        si = drain_inst.ins.sync_info
        waits = list(si.on_wait or [])
        if len(waits) > _MAX_DRAIN_WAITS:
            si.on_wait = waits[:_MAX_DRAIN_WAITS]
            rest = waits[_MAX_DRAIN_WAITS:]
            for i in range(0, len(rest), _MAX_DRAIN_WAITS):
                nop = nc.sync.nop(nofuse=True)
                nop.ins.sync_info = mybir.SyncInfo(
                    on_wait=rest[i : i + _MAX_DRAIN_WAITS], on_update=[]
                )
        nc.all_engine_barrier()
        assert self.sems is not None
        popped = nc._tile_sem_poison_stack.pop()
        assert popped is self._sem_poison
        nc.clear_and_free_semaphores(list(self.sems.allocated().values()))
        nc.all_engine_barrier()

    tile.TileContext._drain_and_barrier = _drain_and_barrier
    tile.TileContext._drain_patched = True


def build_nc(split_waits=True):
    _patch_tail_drain()
    nc = bass.Bass("TRN2", target_bir_lowering=False, debug=False)

    x_d = nc.declare_dram_parameter("x", [C, L], BF16, isOutput=False)
    xf8_d = nc.declare_dram_parameter("xf8", [C, L], F8, isOutput=False)
    w1qk_d = nc.declare_dram_parameter("w1qk", [128, 2 * 384], F8, isOutput=False)
    w1tv_d = nc.declare_dram_parameter("w1tv", [C, C], BF16, isOutput=False)
    wqk8_d = nc.declare_dram_parameter("wqk8", [128, 30 * 128], F8, isOutput=False)
    w3_d = nc.declare_dram_parameter("w3", [128, 9 * 128], BF16, isOutput=False)
    w4_d = nc.declare_dram_parameter("w4", [128, 6 * 64], BF16, isOutput=False)
    w4s_d = nc.declare_dram_parameter("w4s", [64, 9 * 64], BF16, isOutput=False)
    wqs_d = nc.declare_dram_parameter("wqs", [128, 27 * 128], BF16, isOutput=False)
    woutt_d = nc.declare_dram_parameter("woutt", [C, C], F32, isOutput=False)
    y_d = nc.declare_dram_parameter("y", [C, L], F32, isOutput=True)

    with tile.TileContext(nc) as tc, ExitStack() as ctx:
        _body(ctx, tc, x_d, xf8_d, w1qk_d, w1tv_d, wqk8_d, w3_d, w4_d, w4s_d, wqs_d, woutt_d, y_d)
    if split_waits:
        _split_excess_waits(nc)
    return nc


def _split_excess_waits(nc, maxw=1):
    """This walrus build accepts only one semaphore wait per instruction.
    Move excess waits onto same-engine no-ops inserted just before the
    offending instruction (same-engine program order preserves semantics)."""
    uid = [0]
    for f in nc.m.functions:
        for bb in f.blocks:
            il = bb.instructions
            out = []
            changed = False
            for inst in il:
                si = inst.sync_info
                waits = list(si.on_wait) if si and si.on_wait else []
                if len(waits) > maxw:
                    changed = True
                    rest, keep = waits[:-maxw], waits[-maxw:]
                    for i in range(0, len(rest), maxw):
                        uid[0] += 1
                        out.append(
                            mybir.InstNoOp(
                                name=f"I-waitsplit-{uid[0]}",
                                engine=inst.engine,
                                ins=[],
                                outs=[],
                                sync_info=mybir.SyncInfo(
                                    on_wait=rest[i : i + maxw], on_update=[]
                                ),
                            )
                        )
                    si.on_wait = keep
                out.append(inst)
            if changed:
                bb.instructions = out


def _tap_pair_ap(ztile, orow, p):
    """rhs AP [128, 2(k-tiles), 4(rows), W] for dw tap pair p over a
    [128, ZROWS, PADW] z tile; k-tile t reads tap TAPS[2p+t]."""
    t0 = TAPS[2 * p]
    off0 = (orow + t0[0]) * PADW + (1 + t0[1])
    if 2 * p + 1 < 9:
        t1 = TAPS[2 * p + 1]
        delta = (orow + t1[0]) * PADW + (1 + t1[1]) - off0
    else:
        delta = 0  # pad pair: zero weights, reread tap t0 (stride-0 is legal)
    base = ztile[:]
    pstride = base.ap[0][0]
    return cap.AP(
        base.tensor,
        base.offset + off0,
        [[pstride, 128], [delta, 2], [PADW, 4], [1, W]],
    )


def _body(ctx, tc, x_d, xf8_d, w1qk_d, w1tv_d, wqk8_d, w3_d, w4_d, w4s_d, wqs_d, woutt_d, y_d):
    nc = tc.nc
    ncopy = [0]

    def copy(dst, src):
        # alternate PSUM->SBUF copies between ACT and DVE
        if ncopy[0] % 2 == 0:
            nc.scalar.copy(dst, src)
        else:
            nc.vector.tensor_copy(dst, src)
        ncopy[0] += 1

    const = ctx.enter_context(tc.tile_pool(name="const", bufs=1))
    persist = ctx.enter_context(tc.tile_pool(name="persist", bufs=1))

    # ---- constants / weights ----
    w1qk = const.tile([128, 2, 384], F8, tag="w1qk")
    nc.sync.dma_start(w1qk[:], w1qk_d[:].rearrange("k (t m) -> k t m", t=2))
    wqk8 = const.tile([128, 30, 128], F8, tag="wqk8")
    nc.sync.dma_start(wqk8[:], wqk8_d[:].rearrange("k (p m) -> k p m", p=30))
    w3 = const.tile([128, 9, 128], BF16, tag="w3")
    nc.sync.dma_start(w3[:], w3_d[:].rearrange("k (t m) -> k t m", t=9))
    w4 = const.tile([128, 6, 64], BF16, tag="w4")
    nc.sync.dma_start(w4[:], w4_d[:].rearrange("k (t m) -> k t m", t=6))
    w4s = const.tile([64, 9, 64], BF16, tag="w4s")
    nc.sync.dma_start(w4s[:], w4s_d[:].rearrange("k (t m) -> k t m", t=9))
    if not FP8_DW:
        wqs = const.tile([128, 27, 128], BF16, tag="wqs")
        nc.sync.dma_start(wqs[:], wqs_d[:].rearrange("k (t m) -> k t m", t=27))

    w1tv0 = const.tile([128, C], BF16, tag="w1tv0")
    w1tv1 = const.tile([128, C], BF16, tag="w1tv1")
    nc.gpsimd.memset(w1tv1[:], 0.0)
    nc.sync.dma_start(w1tv0[:], w1tv_d[0:128, :])
    nc.sync.dma_start(w1tv1[0:64, :], w1tv_d[128:192, :])

    woutt0 = const.tile([128, C], F32, tag="woutt0")
    woutt1 = const.tile([64, C], F32, tag="woutt1")
    nc.sync.dma_start(woutt0[:], woutt_d[0:128, :])
    nc.sync.dma_start(woutt1[:], woutt_d[128:192, :])
    woutt0_bf = const.tile([128, C], BF16, tag="woutt0bf")
    woutt1_bf = const.tile([64, C], BF16, tag="woutt1bf")
    nc.vector.tensor_copy(woutt0_bf[:], woutt0[:])
    nc.vector.tensor_copy(woutt1_bf[:], woutt1[:])

    ident48 = const.tile([48, 48], F32, tag="ident48")
    make_identity(nc, ident48[:])
    ones48 = const.tile([48, 1], F32, tag="ones48")
    nc.gpsimd.memset(ones48[:], 1.0)
    ones1x48 = const.tile([1, 48], F32, tag="ones1x48")
    nc.gpsimd.memset(ones1x48[:], 1.0)

    # ---- persistent state ----
    v0 = persist.tile([128, L], BF16, tag="v0")
    v1 = persist.tile([128, L], BF16, tag="v1")
    nc.gpsimd.memset(v1[64:128, :], 0.0)
    ZQDT = F8 if FP8_DW else BF16
    zqk = [
        [
            persist.tile([128, ZROWS, PADW], ZQDT, tag=f"zqk{s}_{b}", name=f"zqk{s}_{b}")
            for b in range(3)
        ]
        for s in range(2)
    ]
    zv3 = [persist.tile([128, ZROWS, PADW], BF16, tag=f"zv3_{s}", name=f"zv3_{s}") for s in range(2)]
    zv4 = [persist.tile([128, ZROWS, PADW], BF16, tag=f"zv4_{s}", name=f"zv4_{s}") for s in range(2)]
    for s in range(2):
        for b in range(3):
            nc.gpsimd.memset(zqk[s][b][:], 0.0)
        nc.gpsimd.memset(zv3[s][:], 0.0)
        nc.gpsimd.memset(zv4[s][:], 0.0)

    ghs = persist.tile([48, HEADS * 144], F32, tag="ghs")
    xt1_pp = [persist.tile([128, ZROWS, W], BF16, tag=f"xt1_{s}", name=f"xt1_{s}") for s in range(2)]
    xf8_pp = [
        persist.tile([128, 2, ZROWS, W], F8, tag=f"xf8_{s}", name=f"xf8_{s}") for s in range(2)
    ]
    for s in range(2):
        nc.gpsimd.memset(xt1_pp[s][:], 0.0)
        nc.gpsimd.memset(xf8_pp[s][:], 0.0)

    # ---- phase A: chunked pipeline ----
    with (
        tc.tile_pool(name="gps", bufs=1, space="PSUM") as gps,
        tc.tile_pool(name="xp", bufs=2) as xp,
        tc.tile_pool(name="zps", bufs=3, space="PSUM") as zps,
        tc.tile_pool(name="qps", bufs=2, space="PSUM") as qps,
        tc.tile_pool(name="stp", bufs=2) as stp,
        tc.tile_pool(name="qktp", bufs=2) as qktp,
    ):
        # two G banks; a single accumulation group spans all heads per bank
        g1 = gps.tile([48, HEADS * 96], F32, tag="g1")
        g2 = gps.tile([48, HEADS * 48], F32, tag="g2")
        for c in range(NCHUNK):
            zq = zqk[c % 2]
            z3 = zv3[c % 2]
            z4 = zv4[c % 2]
            r0 = max(0, R * c - 1)
            r1 = min(H, R * c + R + 1)
            nrows = r1 - r0
            brow0 = r0 - (R * c - 1)  # buf row of image row r0

            xt0 = xp.tile([128, nrows, W], BF16, tag="x0")
            xt1 = xt1_pp[c % 2]
            xf8 = xf8_pp[c % 2]
            nc.sync.dma_start(
                xt0[:], x_d[0:128, r0 * W : r1 * W].rearrange("p (r w) -> p r w", w=W)
            )
            nc.sync.dma_start(
                xt1[0:64, 0:nrows, :],
                x_d[128:192, r0 * W : r1 * W].rearrange("p (r w) -> p r w", w=W),
            )
            nc.sync.dma_start(
                xf8[:, 0, 0:nrows, :],
                xf8_d[0:128, r0 * W : r1 * W].rearrange("p (r w) -> p r w", w=W),
            )
            nc.sync.dma_start(
                xf8[0:64, 1, 0:nrows, :],
                xf8_d[128:192, r0 * W : r1 * W].rearrange("p (r w) -> p r w", w=W),
            )

            # conv1 into padded z tiles (groups of <=4 rows)
            for g0 in range(0, nrows, 4):
                gn = min(4, nrows - g0)
                rsl = slice(brow0 + g0, brow0 + g0 + gn)
                for b in range(3):
                    ps = zps.tile([128, 512], F32, tag="zps")
                    nc.tensor.matmul(
                        ps[:, 0 : gn * W],
                        w1qk[:, :, b * 128 : (b + 1) * 128],
                        xf8[:, :, g0 : g0 + gn, :],
                        start=True,
                        stop=True,
                        perf_mode=DR,
                    )
                    copy(zq[b][:, rsl, 1 : 1 + W], ps[:, 0 : gn * W])
                ps3 = zps.tile([128, 512], F32, tag="zps")
                nc.tensor.matmul(
                    ps3[:, 0 : gn * W], w1tv0[:, 0:128], xt0[:, g0 : g0 + gn, :],
                    start=True, stop=False,
                )
                nc.tensor.matmul(
                    ps3[:, 0 : gn * W], w1tv1[:, 0:128], xt1[:, g0 : g0 + gn, :],
                    start=False, stop=True,
                )
                copy(z3[:, rsl, 1 : 1 + W], ps3[:, 0 : gn * W])
                ps4 = zps.tile([128, 512], F32, tag="zps")
                nc.tensor.matmul(
                    ps4[0:64, 0 : gn * W], w1tv0[:, 128:192], xt0[:, g0 : g0 + gn, :],
                    start=True, stop=False,
                )
                nc.tensor.matmul(
                    ps4[0:64, 0 : gn * W], w1tv1[:, 128:192], xt1[:, g0 : g0 + gn, :],
                    start=False, stop=True,
                )
                copy(z4[0:64, rsl, 1 : 1 + W], ps4[0:64, 0 : gn * W])
                if BLK4_PAIR:
                    # partition-shifted duplicate (col -1) for the dw tap-pair
                    # trick; engines can't cross partitions, so DMA the bf16 copy
                    nc.scalar.dma_start(z4[64:128, rsl, 0:W], z4[0:64, rsl, 1 : 1 + W])

            if c == NCHUNK - 1:
                # bottom halo row never written this chunk; clear stale data
                for b in range(3):
                    nc.gpsimd.memset(zq[b][:, ZROWS - 1 : ZROWS, :], 0.0)
                nc.gpsimd.memset(z3[:, ZROWS - 1 : ZROWS, :], 0.0)
                nc.gpsimd.memset(z4[:, ZROWS - 1 : ZROWS, :], 0.0)

            # dw taps -> qkv rows Rc..Rc+R
            st = [stp.tile([128, R // 4, 4 * W], BF16, tag=f"st{i}", name=f"st{i}") for i in range(3)]
            for g in range(R // 4):
                orow = 1 + 4 * g  # buf row of first output row in this group
                for b in range(3):
                    ps = qps.tile([128, 512], F32, tag="qps")
                    if FP8_DW == 1:
                        for p in range(5):
                            nc.tensor.matmul(
                                ps[:, :],
                                wqk8[:, (b * 5 + p) * 2 : (b * 5 + p) * 2 + 2, :],
                                _tap_pair_ap(zq[b], orow, p),
                                start=(p == 0),
                                stop=(p == 4),
                                perf_mode=DR,
                            )
                    elif FP8_DW == 2:
                        for t, (di, dj) in enumerate(TAPS):
                            nc.tensor.matmul(
                                ps[:, :],
                                wqk8[:, (b * 5 + t // 2) * 2 + t % 2, :],
                                zq[b][:, orow + di : orow + di + 4, 1 + dj : 1 + dj + W],
                                start=(t == 0),
                                stop=(t == 8),
                            )
                    else:
                        for t, (di, dj) in enumerate(TAPS):
                            nc.tensor.matmul(
                                ps[:, :],
                                wqs[:, b * 9 + t, :],
                                zq[b][:, orow + di : orow + di + 4, 1 + dj : 1 + dj + W],
                                start=(t == 0),
                                stop=(t == 8),
                            )
                    copy(st[b][:, g, :], ps[:, :])
                ps3 = qps.tile([128, 512], F32, tag="qps")
                for t, (di, dj) in enumerate(TAPS):
                    nc.tensor.matmul(
                        ps3[:, :],
                        w3[:, t, :],
                        z3[:, orow + di : orow + di + 4, 1 + dj : 1 + dj + W],
                        start=(t == 0),
                        stop=(t == 8),
                    )
                copy(v0[:, c * R * W + g * 512 : c * R * W + (g + 1) * 512], ps3[:, :])
                ps4 = qps.tile([64, 512], F32, tag="qps4", bufs=1)
                if BLK4_PAIR:
                    for i, di in enumerate((-1, 0, 1)):
                        nc.tensor.matmul(
                            ps4[:, :],
                            w4[:, 2 * i, :],
                            z4[:, orow + di : orow + di + 4, 0:W],
                            start=(i == 0),
                            stop=False,
                        )
                        nc.tensor.matmul(
                            ps4[:, :],
                            w4[0:64, 2 * i + 1, :],
                            z4[0:64, orow + di : orow + di + 4, 2 : 2 + W],
                            start=False,
                            stop=(i == 2),
                        )
                else:
                    for t, (di, dj) in enumerate(TAPS):
                        nc.tensor.matmul(
                            ps4[:, :],
                            w4s[0:64, t, :],
                            z4[0:64, orow + di : orow + di + 4, 1 + dj : 1 + dj + W],
                            start=(t == 0),
                            stop=(t == 8),
                        )
                copy(v1[0:64, c * R * W + g * 512 : c * R * W + (g + 1) * 512], ps4[:, :])

            # transpose q,k: qkt[:, lt, 0, :] = k^T, [:, lt, 1, :] = q^T
            st_flat = [s.rearrange("p a b -> p (a b)") for s in st]
            qkt = qktp.tile([128, R, 2, 192], BF16, tag="qkt")
            nc.sync.dma_start_transpose(qkt[:, :, 1, 0:128], st_flat[0][:, :])
            nc.scalar.dma_start_transpose(qkt[:, :, 1, 128:192], st_flat[1][0:64, :])
            nc.sync.dma_start_transpose(qkt[:, :, 0, 0:64], st_flat[1][64:128, :])
            nc.scalar.dma_start_transpose(qkt[:, :, 0, 64:192], st_flat[2][:, :])

            # gram accumulation
            for lt in range(R):
                first = c == 0 and lt == 0
                last = c == NCHUNK - 1 and lt == R - 1
                for h in range(HEADS):
                    nc.tensor.matmul(
                        g1[:, h * 96 : h * 96 + 96],
                        qkt[:, lt, 1, h * DH : (h + 1) * DH],
                        qkt[:, lt, :, h * DH : (h + 1) * DH],
                        start=(first and h == 0),
                        stop=(last and h == HEADS - 1),
                        skip_group_check=True,
                    )
                    nc.tensor.matmul(
                        g2[:, h * DH : (h + 1) * DH],
                        qkt[:, lt, 0, h * DH : (h + 1) * DH],
                        qkt[:, lt, 0, h * DH : (h + 1) * DH],
                        start=(first and h == 0),
                        stop=(last and h == HEADS - 1),
                        skip_group_check=True,
                    )

        nc.vector.tensor_copy(ghs[:, 0 : HEADS * 96], g1[:])
        nc.vector.tensor_copy(ghs[:, HEADS * 96 :], g2[:])

    # ---- phase B ----
    with (
        tc.tile_pool(name="bsb", bufs=1) as bsb,
        tc.tile_pool(name="bps", bufs=1, space="PSUM") as bps,
        tc.tile_pool(name="ops", bufs=4, space="PSUM") as ops,
        tc.tile_pool(name="osb", bufs=4) as osb,
    ):
        attn_bf = bsb.tile([48, HEADS * 48], BF16, tag="attnbf")
        scr = bsb.tile([48, 48], F32, tag="scr")
        scr2 = bsb.tile([48, 48], F32, tag="scr2")
        colv = bsb.tile([48, 1], F32, tag="colv")
        rowv = bsb.tile([1, 48], F32, tag="rowv")
        rkrep = bsb.tile([48, 48], F32, tag="rkrep")
        logits = bsb.tile([48, 48], F32, tag="logits")

        for h in range(HEADS):
            gqk = ghs[:, h * 96 : h * 96 + 48]
            gqq = ghs[:, h * 96 + 48 : h * 96 + 96]
            gkk = ghs[:, HEADS * 96 + h * DH : HEADS * 96 + (h + 1) * DH]

            # rq_inv = 1/max(sqrt(diag(Gqq)),eps); 1/sqrt(L) scale is constant
            # across the softmax row only if folded for both q and k norms
            nc.vector.tensor_mul(scr[:], gqq, ident48[:])
            nc.vector.reduce_sum(colv[:], scr[:], axis=mybir.AxisListType.X)
            nc.scalar.activation(colv[:], colv[:], AF.Sqrt)
            nc.vector.tensor_scalar_max(colv[:], colv[:], 1e-12)
            nc.vector.reciprocal(colv[:], colv[:])
            nc.vector.tensor_scalar(
                logits[:],
                gqk,
                colv[:],
                1.0 / math.sqrt(L),
                op0=mybir.AluOpType.mult,
                op1=mybir.AluOpType.mult,
            )

            # rk_inv broadcast along the free (key) dim via diag-as-row
            nc.vector.tensor_mul(scr2[:], gkk, ident48[:])
            ps_row = bps.tile([1, 48], F32, tag="pssmall")
            nc.tensor.matmul(ps_row[:], ones48[:], scr2[:], start=True, stop=True)
            nc.vector.tensor_copy(rowv[:], ps_row[:])
            nc.scalar.activation(rowv[:], rowv[:], AF.Sqrt)
            nc.vector.tensor_scalar_max(rowv[:], rowv[:], 1e-12)
            nc.vector.reciprocal(rowv[:], rowv[:])
            ps_rep = bps.tile([48, 48], F32, tag="pssmall")
            nc.tensor.matmul(ps_rep[:], ones1x48[:], rowv[:], start=True, stop=True)
            nc.vector.tensor_copy(rkrep[:], ps_rep[:])
            nc.vector.tensor_mul(logits[:], logits[:], rkrep[:])

            # softmax over the free (key) dim
            nc.vector.reduce_max(colv[:], logits[:], axis=mybir.AxisListType.X)
            nc.vector.tensor_scalar_sub(logits[:], logits[:], colv[:])
            nc.scalar.activation(logits[:], logits[:], AF.Exp)
            nc.vector.reduce_sum(colv[:], logits[:], axis=mybir.AxisListType.X)
            nc.vector.reciprocal(colv[:], colv[:])
            nc.vector.tensor_scalar_mul(logits[:], logits[:], colv[:])
            nc.vector.tensor_copy(attn_bf[:, h * 48 : (h + 1) * 48], logits[:])

        # block-diagonal attn (bf16)
        bd0 = bsb.tile([128, C], BF16, tag="bd0")
        bd1 = bsb.tile([64, C], BF16, tag="bd1")
        nc.gpsimd.memset(bd0[:], 0.0)
        nc.gpsimd.memset(bd1[:], 0.0)
        nc.sync.dma_start(bd0[0:48, 0:48], attn_bf[:, 0:48])
        nc.sync.dma_start(bd0[48:96, 48:96], attn_bf[:, 48:96])
        nc.sync.dma_start(bd0[96:128, 96:144], attn_bf[0:32, 96:144])
        nc.sync.dma_start(bd1[0:16, 96:144], attn_bf[32:48, 96:144])
        nc.sync.dma_start(bd1[16:64, 144:192], attn_bf[:, 144:192])

        # W_effT = BD(attn).T @ W_outT   [192 x 192], bf16
        weff0 = bsb.tile([128, 256], BF16, tag="weff0")
        weff1 = bsb.tile([128, 256], BF16, tag="weff1")
        nc.gpsimd.memset(weff0[:], 0.0)
        nc.gpsimd.memset(weff1[:], 0.0)
        for m0, m1, wt in [(0, 128, weff0), (128, 192, weff1)]:
            pw = bps.tile([128, C], F32, tag="pweff")
            nc.tensor.matmul(pw[0 : m1 - m0, :], bd0[:, m0:m1], woutt0_bf[:], start=True, stop=False)
            nc.tensor.matmul(pw[0 : m1 - m0, :], bd1[:, m0:m1], woutt1_bf[:], start=False, stop=True)
            copy(wt[0 : m1 - m0, 0:C], pw[0 : m1 - m0, :])

        # y = W_effT.T @ v
        for g in range(L // 512):
            sl = slice(g * 512, (g + 1) * 512)
            for m0, m1 in [(0, 128), (128, 192)]:
                po = ops.tile([128, 512], F32, tag="ops")
                nc.tensor.matmul(po[:, :], weff0[:, m0 : m0 + 128], v0[:, sl], start=True, stop=False)
                nc.tensor.matmul(po[:, :], weff1[:, m0 : m0 + 128], v1[:, sl], start=False, stop=True)
                ot = osb.tile([m1 - m0, 512], F32, tag=f"o{m0}", name=f"o{m0}")
                copy(ot[:], po[0 : m1 - m0, :])
                nc.sync.dma_start(y_d[m0:m1, sl], ot[:])


def _tap_idx(di, dj):
    return 3 * (di + 1) + (dj + 1)


def _prep_weights(w_proj1, w_dw, w_out):
    import ml_dtypes

    E4 = ml_dtypes.float8_e4m3
    w1t = np.asarray(w_proj1, np.float32).reshape(CQKV, C).T  # [in, out]
    wdw = np.asarray(w_dw, np.float32).reshape(CQKV, 9)
    woutt = np.ascontiguousarray(np.asarray(w_out, np.float32).reshape(C, C).T)

    w1qk = np.zeros((128, 2, 384), np.float32)
    w1qk[:, 0, :] = w1t[0:128, 0:384] * S1
    w1qk[0:64, 1, :] = w1t[128:192, 0:384] * S1
    w1qk = np.ascontiguousarray(w1qk.reshape(128, 768)).astype(E4)

    w1tv = np.ascontiguousarray(w1t[:, 384:576]).astype(ml_dtypes.bfloat16)

    wqk8 = np.zeros((128, 30, 128), np.float32)
    for b in range(3):
        for p in range(5):
            for s in range(2):
                t = 2 * p + s
                if t < 9:
                    np.fill_diagonal(wqk8[:, (b * 5 + p) * 2 + s, :], wdw[b * 128 : (b + 1) * 128, t] * S2)
    wqk8 = np.ascontiguousarray(wqk8.reshape(128, 3840)).astype(E4)

    w3 = np.zeros((128, 9, 128), np.float32)
    for t in range(9):
        np.fill_diagonal(w3[:, t, :], wdw[384:512, t])
    w3 = np.ascontiguousarray(w3.reshape(128, 1152)).astype(ml_dtypes.bfloat16)

    w4 = np.zeros((128, 6, 64), np.float32)
    for i, di in enumerate((-1, 0, 1)):
        np.fill_diagonal(w4[0:64, 2 * i, :], wdw[512:576, _tap_idx(di, -1)])
        np.fill_diagonal(w4[64:128, 2 * i, :], wdw[512:576, _tap_idx(di, 0)])
        np.fill_diagonal(w4[0:64, 2 * i + 1, :], wdw[512:576, _tap_idx(di, 1)])
    w4 = np.ascontiguousarray(w4.reshape(128, 384)).astype(ml_dtypes.bfloat16)

    w4s = np.zeros((64, 9, 64), np.float32)
    for t in range(9):
        np.fill_diagonal(w4s[:, t, :], wdw[512:576, t])
    w4s = np.ascontiguousarray(w4s.reshape(64, 576)).astype(ml_dtypes.bfloat16)

    wqs = np.zeros((128, 27, 128), np.float32)
    for b in range(3):
        for t in range(9):
            np.fill_diagonal(wqs[:, b * 9 + t, :], wdw[b * 128 : (b + 1) * 128, t])
    wqs = np.ascontiguousarray(wqs.reshape(128, 27 * 128)).astype(ml_dtypes.bfloat16)

    return {
        "w1qk": w1qk,
        "w1tv": w1tv,
        "wqk8": wqk8,
        "w3": w3,
        "w4": w4,
        "w4s": w4s,
        "wqs": wqs,
        "woutt": woutt,
    }


_NC_CACHE = None


def _get_nc():
    global _NC_CACHE
    if _NC_CACHE is None:
        _NC_CACHE = build_nc()
    return _NC_CACHE


def kernel(x, w_proj1, w_dw, pos_emb, w_out, _trace=False):
    from concourse.bass_utils import run_bass_kernel_spmd

    import ml_dtypes

    xf = np.asarray(x, dtype=np.float32)
    xbf = xf.astype(ml_dtypes.bfloat16)
    xf8 = xf.astype(ml_dtypes.float8_e4m3)
    wmaps = _prep_weights(w_proj1, w_dw, w_out)
    # pos_emb adds a per-head constant to every logit in its softmax row;
    # softmax is shift-invariant, so it has no effect on the output.

    nc = _get_nc()
    in_maps = [
        {
            "x": np.ascontiguousarray(xbf[b].reshape(C, L)),
            "xf8": np.ascontiguousarray(xf8[b].reshape(C, L)),
            **wmaps,
        }
        for b in range(N_CORES)
    ]
    res = run_bass_kernel_spmd(nc, in_maps, list(range(N_CORES)), trace=_trace)
    out = np.stack([res.results[b]["y"].reshape(C, H, W) for b in range(N_CORES)])
    if _trace:
        kernel.last_exec_time_ns = res.exec_time_ns
        kernel.last_profile = res
    return out.astype(np.float32)


# revision 13
# speedup vs baseline: 1.0232x; 1.0207x over previous
"""ChannelSA Trainium2 kernel: 8-way batch-parallel across NeuronCores.

kernel(**inputs) takes the FULL inputs (x [8,192,128,128], conv weights,
pos_emb) and returns the FULL output [8,192,128,128] fp32. Each core runs
an identical single-batch program (SPMD, no collectives).

Per-core pipeline (v2: fp8 DoubleRow on the q,k path):
  q,k path is scale-invariant (logits are the normalized Gram
  Gqk/(|q||k|), softmax is shift-invariant) so it runs entirely in
  fp8e4m3 with prescaled weights at the PE's DoubleRow rate (2 K-tiles
  per pass, 0.5 cyc/col):
    zqk = W1qk @ x        one DR matmul per 128-out block (K=192 in 2 k-tiles)
    q,k = DW3x3(zqk)      9 taps as 5 DR diag-pair matmuls per block
  v path stays bf16 for accuracy:
    zv  = W1v @ x         K=128+64 accumulating matmuls
    v   = DW3x3(zv)       block3: 9 diag matmuls; block4 (64ch): z stored
                          twice (partitions 64:128 col-shifted) so taps
                          (di,-1)+(di,0) share one matmul -> 6 matmuls
  q,k -> bf16 DMA-transpose -> per-head Gram banks [Gqk|Gqq|Gkk] in PSUM
  logits = Gqk / (|q||k| sqrt(L))  (pos_emb is a per-row constant: no-op)
  attn = softmax(logits)
  y = (W_out @ blockdiag(attn)) @ v
"""
import math
from contextlib import ExitStack

import numpy as np

import concourse.ap as cap
import concourse.bass as bass
import concourse.mybir as mybir
import concourse.tile as tile
from concourse.masks import make_identity

F32 = mybir.dt.float32
BF16 = mybir.dt.bfloat16
F8 = mybir.dt.float8e4
AF = mybir.ActivationFunctionType
DR = mybir.MatmulPerfMode.DoubleRow

C = 192
CQKV = 576
H = 128
W = 128
L = H * W
HEADS = 4
DH = 48
R = 8                    # output image rows per chunk
NCHUNK = H // R
PADW = W + 2             # padded row stride in z tiles
ZROWS = R + 2            # rows held per z chunk (1 halo each side)
TAPS = [(di, dj) for di in (-1, 0, 1) for dj in (-1, 0, 1)]
N_CORES = 8
BLK4_PAIR = True        # block4 dw tap-pair trick (partition dup)
FP8_DW = 2               # q,k dw mode: 0=bf16 diag, 1=fp8 DoubleRow pairs, 2=fp8 singles
S1 = 16.0                # fp8 prescale on W1 qk columns
S2 = 4.0                 # fp8 prescale on dw qk weights

_MAX_DRAIN_WAITS = 1


def _patch_tail_drain():
    """The walrus in this image rejects >1 semaphore wait on the Tile tail
    drain instruction; split the waits across a chain of SP nops."""
    if getattr(tile.TileContext, "_drain_patched", False):
        return

    def _drain_and_barrier(self, tick_clock, wait_clock):
        from concourse.vector_clock import ScopedClock

        nc = self.nc
        drain_inst = nc.sync.drain()
        wait_clock.add_sem_waits(
            drain_inst.ins, ScopedClock({None: tick_clock.global_clock})
        )
        si = drain_inst.ins.sync_info
        waits = list(si.on_wait or [])
        if len(waits) > _MAX_DRAIN_WAITS:
            si.on_wait = waits[:_MAX_DRAIN_WAITS]
            rest = waits[_MAX_DRAIN_WAITS:]
            for i in range(0, len(rest), _MAX_DRAIN_WAITS):
                nop = nc.sync.nop(nofuse=True)
                nop.ins.sync_info = mybir.SyncInfo(
                    on_wait=rest[i : i + _MAX_DRAIN_WAITS], on_update=[]
                )
        nc.all_engine_barrier()
        assert self.sems is not None
        popped = nc._tile_sem_poison_stack.pop()
        assert popped is self._sem_poison
        nc.clear_and_free_semaphores(list(self.sems.allocated().values()))
        nc.all_engine_barrier()

    tile.TileContext._drain_and_barrier = _drain_and_barrier
    tile.TileContext._drain_patched = True


def build_nc(split_waits=True):
    _patch_tail_drain()
    nc = bass.Bass("TRN2", target_bir_lowering=False, debug=False)

    x_d = nc.declare_dram_parameter("x", [C, L], BF16, isOutput=False)
    xf8_d = nc.declare_dram_parameter("xf8", [C, L], F8, isOutput=False)
    w1qk_d = nc.declare_dram_parameter("w1qk", [128, 2 * 384], F8, isOutput=False)
    w1tv_d = nc.declare_dram_parameter("w1tv", [C, C], BF16, isOutput=False)
    wqk8_d = nc.declare_dram_parameter("wqk8", [128, 30 * 128], F8, isOutput=False)
    w3_d = nc.declare_dram_parameter("w3", [128, 9 * 128], BF16, isOutput=False)
    w4_d = nc.declare_dram_parameter("w4", [128, 6 * 64], BF16, isOutput=False)
    w4s_d = nc.declare_dram_parameter("w4s", [64, 9 * 64], BF16, isOutput=False)
    wqs_d = nc.declare_dram_parameter("wqs", [128, 27 * 128], BF16, isOutput=False)
    woutt_d = nc.declare_dram_parameter("woutt", [C, C], F32, isOutput=False)
    y_d = nc.declare_dram_parameter("y", [C, L], F32, isOutput=True)

    with tile.TileContext(nc) as tc, ExitStack() as ctx:
        _body(ctx, tc, x_d, xf8_d, w1qk_d, w1tv_d, wqk8_d, w3_d, w4_d, w4s_d, wqs_d, woutt_d, y_d)
    if split_waits:
        _split_excess_waits(nc)
    return nc


def _split_excess_waits(nc, maxw=1):
    """This walrus build accepts only one semaphore wait per instruction.
    Move excess waits onto same-engine no-ops inserted just before the
    offending instruction (same-engine program order preserves semantics)."""
    uid = [0]
    for f in nc.m.functions:
        for bb in f.blocks:
            il = bb.instructions
            out = []
            changed = False
            for inst in il:
                si = inst.sync_info
                waits = list(si.on_wait) if si and si.on_wait else []
                if len(waits) > maxw:
                    changed = True
                    rest, keep = waits[:-maxw], waits[-maxw:]
                    for i in range(0, len(rest), maxw):
                        uid[0] += 1
                        out.append(
                            mybir.InstNoOp(
                                name=f"I-waitsplit-{uid[0]}",
                                engine=inst.engine,
                                ins=[],
                                outs=[],
                                sync_info=mybir.SyncInfo(
                                    on_wait=rest[i : i + maxw], on_update=[]
                                ),
                            )
                        )
                    si.on_wait = keep
                out.append(inst)
            if changed:
                bb.instructions = out


def _tap_pair_ap(ztile, orow, p):
    """rhs AP [128, 2(k-tiles), 4(rows), W] for dw tap pair p over a
    [128, ZROWS, PADW] z tile; k-tile t reads tap TAPS[2p+t]."""
    t0 = TAPS[2 * p]
    off0 = (orow + t0[0]) * PADW + (1 + t0[1])
    if 2 * p + 1 < 9:
        t1 = TAPS[2 * p + 1]
        delta = (orow + t1[0]) * PADW + (1 + t1[1]) - off0
    else:
        delta = 0  # pad pair: zero weights, reread tap t0 (stride-0 is legal)
    base = ztile[:]
    pstride = base.ap[0][0]
    return cap.AP(
        base.tensor,
        base.offset + off0,
        [[pstride, 128], [delta, 2], [PADW, 4], [1, W]],
    )


def _body(ctx, tc, x_d, xf8_d, w1qk_d, w1tv_d, wqk8_d, w3_d, w4_d, w4s_d, wqs_d, woutt_d, y_d):
    nc = tc.nc
    ncopy = [0]

    def copy(dst, src):
        # alternate PSUM->SBUF copies between ACT and DVE
        if ncopy[0] % 2 == 0:
            nc.scalar.copy(dst, src)
        else:
            nc.vector.tensor_copy(dst, src)
        ncopy[0] += 1

    const = ctx.enter_context(tc.tile_pool(name="const", bufs=1))
    persist = ctx.enter_context(tc.tile_pool(name="persist", bufs=1))

    # ---- constants / weights ----
    w1qk = const.tile([128, 2, 384], F8, tag="w1qk")
    nc.sync.dma_start(w1qk[:], w1qk_d[:].rearrange("k (t m) -> k t m", t=2))
    wqk8 = const.tile([128, 30, 128], F8, tag="wqk8")
    nc.sync.dma_start(wqk8[:], wqk8_d[:].rearrange("k (p m) -> k p m", p=30))
    w3 = const.tile([128, 9, 128], BF16, tag="w3")
    nc.sync.dma_start(w3[:], w3_d[:].rearrange("k (t m) -> k t m", t=9))
    w4 = const.tile([128, 6, 64], BF16, tag="w4")
    nc.sync.dma_start(w4[:], w4_d[:].rearrange("k (t m) -> k t m", t=6))
    w4s = const.tile([64, 9, 64], BF16, tag="w4s")
    nc.sync.dma_start(w4s[:], w4s_d[:].rearrange("k (t m) -> k t m", t=9))
    if not FP8_DW:
        wqs = const.tile([128, 27, 128], BF16, tag="wqs")
        nc.sync.dma_start(wqs[:], wqs_d[:].rearrange("k (t m) -> k t m", t=27))

    w1tv0 = const.tile([128, C], BF16, tag="w1tv0")
    w1tv1 = const.tile([128, C], BF16, tag="w1tv1")
    nc.gpsimd.memset(w1tv1[:], 0.0)
    nc.sync.dma_start(w1tv0[:], w1tv_d[0:128, :])
    nc.sync.dma_start(w1tv1[0:64, :], w1tv_d[128:192, :])

    woutt0 = const.tile([128, C], F32, tag="woutt0")
    woutt1 = const.tile([64, C], F32, tag="woutt1")
    nc.sync.dma_start(woutt0[:], woutt_d[0:128, :])
    nc.sync.dma_start(woutt1[:], woutt_d[128:192, :])
    woutt0_bf = const.tile([128, C], BF16, tag="woutt0bf")
    woutt1_bf = const.tile([64, C], BF16, tag="woutt1bf")
    nc.vector.tensor_copy(woutt0_bf[:], woutt0[:])
    nc.vector.tensor_copy(woutt1_bf[:], woutt1[:])

    ident48 = const.tile([48, 48], F32, tag="ident48")
    make_identity(nc, ident48[:])
    ones48 = const.tile([48, 1], F32, tag="ones48")
    nc.gpsimd.memset(ones48[:], 1.0)
    ones1x48 = const.tile([1, 48], F32, tag="ones1x48")
    nc.gpsimd.memset(ones1x48[:], 1.0)

    # ---- persistent state ----
    v0 = persist.tile([128, L], BF16, tag="v0")
    v1 = persist.tile([128, L], BF16, tag="v1")
    nc.gpsimd.memset(v1[64:128, :], 0.0)
    ZQDT = F8 if FP8_DW else BF16
    zqk = [
        [
            persist.tile([128, ZROWS, PADW], ZQDT, tag=f"zqk{s}_{b}", name=f"zqk{s}_{b}")
            for b in range(3)
        ]
        for s in range(2)
    ]
    zv3 = [persist.tile([128, ZROWS, PADW], BF16, tag=f"zv3_{s}", name=f"zv3_{s}") for s in range(2)]
    zv4 = [persist.tile([128, ZROWS, PADW], BF16, tag=f"zv4_{s}", name=f"zv4_{s}") for s in range(2)]
    for s in range(2):
        for b in range(3):
            nc.gpsimd.memset(zqk[s][b][:], 0.0)
        nc.gpsimd.memset(zv3[s][:], 0.0)
        nc.gpsimd.memset(zv4[s][:], 0.0)

    ghs = persist.tile([48, HEADS * 144], F32, tag="ghs")
    xt1_pp = [persist.tile([128, ZROWS, W], BF16, tag=f"xt1_{s}", name=f"xt1_{s}") for s in range(2)]
    xf8_pp = [
        persist.tile([128, 2, ZROWS, W], F8, tag=f"xf8_{s}", name=f"xf8_{s}") for s in range(2)
    ]
    for s in range(2):
        nc.gpsimd.memset(xt1_pp[s][:], 0.0)
        nc.gpsimd.memset(xf8_pp[s][:], 0.0)

    # ---- phase A: chunked pipeline ----
    with (
        tc.tile_pool(name="gps", bufs=1, space="PSUM") as gps,
        tc.tile_pool(name="xp", bufs=2) as xp,
        tc.tile_pool(name="zps", bufs=3, space="PSUM") as zps,
        tc.tile_pool(name="qps", bufs=2, space="PSUM") as qps,
        tc.tile_pool(name="stp", bufs=2) as stp,
        tc.tile_pool(name="qktp", bufs=2) as qktp,
    ):
        # two G banks; a single accumulation group spans all heads per bank
        g1 = gps.tile([48, HEADS * 96], F32, tag="g1")
        g2 = gps.tile([48, HEADS * 48], F32, tag="g2")
        for c in range(NCHUNK):
            zq = zqk[c % 2]
            z3 = zv3[c % 2]
            z4 = zv4[c % 2]
            r0 = max(0, R * c - 1)
            r1 = min(H, R * c + R + 1)
            nrows = r1 - r0
            brow0 = r0 - (R * c - 1)  # buf row of image row r0

            xt0 = xp.tile([128, nrows, W], BF16, tag="x0")
            xt1 = xt1_pp[c % 2]
            xf8 = xf8_pp[c % 2]
            nc.sync.dma_start(
                xt0[:], x_d[0:128, r0 * W : r1 * W].rearrange("p (r w) -> p r w", w=W)
            )
            nc.sync.dma_start(
                xt1[0:64, 0:nrows, :],
                x_d[128:192, r0 * W : r1 * W].rearrange("p (r w) -> p r w", w=W),
            )
            nc.sync.dma_start(
                xf8[:, 0, 0:nrows, :],
                xf8_d[0:128, r0 * W : r1 * W].rearrange("p (r w) -> p r w", w=W),
            )
            nc.sync.dma_start(
                xf8[0:64, 1, 0:nrows, :],
                xf8_d[128:192, r0 * W : r1 * W].rearrange("p (r w) -> p r w", w=W),
            )

            # conv1 into padded z tiles (groups of <=4 rows)
            for g0 in range(0, nrows, 4):
                gn = min(4, nrows - g0)
                rsl = slice(brow0 + g0, brow0 + g0 + gn)
                for b in range(3):
                    ps = zps.tile([128, 512], F32, tag="zps")
                    nc.tensor.matmul(
                        ps[:, 0 : gn * W],
                        w1qk[:, :, b * 128 : (b + 1) * 128],
                        xf8[:, :, g0 : g0 + gn, :],
                        start=True,
                        stop=True,
                        perf_mode=DR,
                    )
                    copy(zq[b][:, rsl, 1 : 1 + W], ps[:, 0 : gn * W])
                ps3 = zps.tile([128, 512], F32, tag="zps")
                nc.tensor.matmul(
                    ps3[:, 0 : gn * W], w1tv0[:, 0:128], xt0[:, g0 : g0 + gn, :],
                    start=True, stop=False,
                )
                nc.tensor.matmul(
                    ps3[:, 0 : gn * W], w1tv1[:, 0:128], xt1[:, g0 : g0 + gn, :],
                    start=False, stop=True,
                )
                copy(z3[:, rsl, 1 : 1 + W], ps3[:, 0 : gn * W])
                ps4 = zps.tile([128, 512], F32, tag="zps")
                nc.tensor.matmul(
                    ps4[0:64, 0 : gn * W], w1tv0[:, 128:192], xt0[:, g0 : g0 + gn, :],
                    start=True, stop=False,
                )
                nc.tensor.matmul(
                    ps4[0:64, 0 : gn * W], w1tv1[:, 128:192], xt1[:, g0 : g0 + gn, :],
                    start=False, stop=True,
                )
                copy(z4[0:64, rsl, 1 : 1 + W], ps4[0:64, 0 : gn * W])
                if BLK4_PAIR:
                    # partition-shifted duplicate (col -1) for the dw tap-pair
                    # trick; engines can't cross partitions, so DMA the bf16 copy
                    nc.scalar.dma_start(z4[64:128, rsl, 0:W], z4[0:64, rsl, 1 : 1 + W])

            if c == NCHUNK - 1:
                # bottom halo row never written this chunk; clear stale data
                for b in range(3):
                    nc.gpsimd.memset(zq[b][:, ZROWS - 1 : ZROWS, :], 0.0)
                nc.gpsimd.memset(z3[:, ZROWS - 1 : ZROWS, :], 0.0)
                nc.gpsimd.memset(z4[:, ZROWS - 1 : ZROWS, :], 0.0)

            # dw taps -> qkv rows Rc..Rc+R
            st = [stp.tile([128, R // 4, 4 * W], BF16, tag=f"st{i}", name=f"st{i}") for i in range(3)]
            for g in range(R // 4):
                orow = 1 + 4 * g  # buf row of first output row in this group
                for b in range(3):
                    ps = qps.tile([128, 512], F32, tag="qps")
                    if FP8_DW == 1:
                        for p in range(5):
                            nc.tensor.matmul(
                                ps[:, :],
                                wqk8[:, (b * 5 + p) * 2 : (b * 5 + p) * 2 + 2, :],
                                _tap_pair_ap(zq[b], orow, p),
                                start=(p == 0),
                                stop=(p == 4),
                                perf_mode=DR,
                            )
                    elif FP8_DW == 2:
                        for t, (di, dj) in enumerate(TAPS):
                            nc.tensor.matmul(
                                ps[:, :],
                                wqk8[:, (b * 5 + t // 2) * 2 + t % 2, :],
                                zq[b][:, orow + di : orow + di + 4, 1 + dj : 1 + dj + W],
                                start=(t == 0),
                                stop=(t == 8),
                            )
                    else:
                        for t, (di, dj) in enumerate(TAPS):
                            nc.tensor.matmul(
                                ps[:, :],
                                wqs[:, b * 9 + t, :],
                                zq[b][:, orow + di : orow + di + 4, 1 + dj : 1 + dj + W],
                                start=(t == 0),
                                stop=(t == 8),
                            )
                    copy(st[b][:, g, :], ps[:, :])
                ps3 = qps.tile([128, 512], F32, tag="qps")
                for t, (di, dj) in enumerate(TAPS):
                    nc.tensor.matmul(
                        ps3[:, :],
                        w3[:, t, :],
                        z3[:, orow + di : orow + di + 4, 1 + dj : 1 + dj + W],
                        start=(t == 0),
                        stop=(t == 8),
                    )
                copy(v0[:, c * R * W + g * 512 : c * R * W + (g + 1) * 512], ps3[:, :])
                ps4 = qps.tile([64, 512], F32, tag="qps4", bufs=1)
                if BLK4_PAIR:
                    for i, di in enumerate((-1, 0, 1)):
                        nc.tensor.matmul(
                            ps4[:, :],
                            w4[:, 2 * i, :],
                            z4[:, orow + di : orow + di + 4, 0:W],
                            start=(i == 0),
                            stop=False,
                        )
                        nc.tensor.matmul(
                            ps4[:, :],
                            w4[0:64, 2 * i + 1, :],
                            z4[0:64, orow + di : orow + di + 4, 2 : 2 + W],
                            start=False,
                            stop=(i == 2),
                        )
                else:
                    for t, (di, dj) in enumerate(TAPS):
                        nc.tensor.matmul(
                            ps4[:, :],
                            w4s[0:64, t, :],
                            z4[0:64, orow + di : orow + di + 4, 1 + dj : 1 + dj + W],
                            start=(t == 0),
                            stop=(t == 8),
                        )
                copy(v1[0:64, c * R * W + g * 512 : c * R * W + (g + 1) * 512], ps4[:, :])

            # transpose q,k: qkt[:, lt, 0, :] = k^T, [:, lt, 1, :] = q^T
            st_flat = [s.rearrange("p a b -> p (a b)") for s in st]
            qkt = qktp.tile([128, R, 2, 192], BF16, tag="qkt")
            nc.sync.dma_start_transpose(qkt[:, :, 1, 0:128], st_flat[0][:, :])
            nc.scalar.dma_start_transpose(qkt[:, :, 1, 128:192], st_flat[1][0:64, :])
            nc.sync.dma_start_transpose(qkt[:, :, 0, 0:64], st_flat[1][64:128, :])
            nc.scalar.dma_start_transpose(qkt[:, :, 0, 64:192], st_flat[2][:, :])

            # gram accumulation
            for lt in range(R):
                first = c == 0 and lt == 0
                last = c == NCHUNK - 1 and lt == R - 1
                for h in range(HEADS):
                    nc.tensor.matmul(
                        g1[:, h * 96 : h * 96 + 96],
                        qkt[:, lt, 1, h * DH : (h + 1) * DH],
                        qkt[:, lt, :, h * DH : (h + 1) * DH],
                        start=(first and h == 0),
                        stop=(last and h == HEADS - 1),
                        skip_group_check=True,
                    )
                    nc.tensor.matmul(
                        g2[:, h * DH : (h + 1) * DH],
                        qkt[:, lt, 0, h * DH : (h + 1) * DH],
                        qkt[:, lt, 0, h * DH : (h + 1) * DH],
                        start=(first and h == 0),
                        stop=(last and h == HEADS - 1),
                        skip_group_check=True,
                    )

        nc.vector.tensor_copy(ghs[:, 0 : HEADS * 96], g1[:])
        nc.vector.tensor_copy(ghs[:, HEADS * 96 :], g2[:])

    # ---- phase B ----
    with (
        tc.tile_pool(name="bsb", bufs=1) as bsb,
        tc.tile_pool(name="bps", bufs=1, space="PSUM") as bps,
        tc.tile_pool(name="ops", bufs=4, space="PSUM") as ops,
        tc.tile_pool(name="osb", bufs=4) as osb,
    ):
        attn_bf = bsb.tile([48, HEADS * 48], BF16, tag="attnbf")
        scr = bsb.tile([48, 48], F32, tag="scr")
        scr2 = bsb.tile([48, 48], F32, tag="scr2")
        colv = bsb.tile([48, 1], F32, tag="colv")
        rowv = bsb.tile([1, 48], F32, tag="rowv")
        rkrep = bsb.tile([48, 48], F32, tag="rkrep")
        logits = bsb.tile([48, 48], F32, tag="logits")

        for h in range(HEADS):
            gqk = ghs[:, h * 96 : h * 96 + 48]
            gqq = ghs[:, h * 96 + 48 : h * 96 + 96]
            gkk = ghs[:, HEADS * 96 + h * DH : HEADS * 96 + (h + 1) * DH]

            # rq_inv = 1/max(sqrt(diag(Gqq)),eps); 1/sqrt(L) scale is constant
            # across the softmax row only if folded for both q and k norms
            nc.vector.tensor_mul(scr[:], gqq, ident48[:])
            nc.vector.reduce_sum(colv[:], scr[:], axis=mybir.AxisListType.X)
            nc.scalar.activation(colv[:], colv[:], AF.Sqrt)
            nc.vector.tensor_scalar_max(colv[:], colv[:], 1e-12)
            nc.vector.reciprocal(colv[:], colv[:])
            nc.vector.tensor_scalar(
                logits[:],
                gqk,
                colv[:],
                1.0 / math.sqrt(L),
                op0=mybir.AluOpType.mult,
                op1=mybir.AluOpType.mult,
            )

            # rk_inv broadcast along the free (key) dim via diag-as-row
            nc.vector.tensor_mul(scr2[:], gkk, ident48[:])
            ps_row = bps.tile([1, 48], F32, tag="pssmall")
            nc.tensor.matmul(ps_row[:], ones48[:], scr2[:], start=True, stop=True)
            nc.vector.tensor_copy(rowv[:], ps_row[:])
            nc.scalar.activation(rowv[:], rowv[:], AF.Sqrt)
            nc.vector.tensor_scalar_max(rowv[:], rowv[:], 1e-12)
            nc.vector.reciprocal(rowv[:], rowv[:])
            ps_rep = bps.tile([48, 48], F32, tag="pssmall")
            nc.tensor.matmul(ps_rep[:], ones1x48[:], rowv[:], start=True, stop=True)
            nc.vector.tensor_copy(rkrep[:], ps_rep[:])
            nc.vector.tensor_mul(logits[:], logits[:], rkrep[:])

            # softmax over the free (key) dim
            nc.vector.reduce_max(colv[:], logits[:], axis=mybir.AxisListType.X)
            nc.vector.tensor_scalar_sub(logits[:], logits[:], colv[:])
            nc.scalar.activation(logits[:], logits[:], AF.Exp)
            nc.vector.reduce_sum(colv[:], logits[:], axis=mybir.AxisListType.X)
            nc.vector.reciprocal(colv[:], colv[:])
            nc.vector.tensor_scalar_mul(logits[:], logits[:], colv[:])
            nc.vector.tensor_copy(attn_bf[:, h * 48 : (h + 1) * 48], logits[:])

        # block-diagonal attn (bf16)
        bd0 = bsb.tile([128, C], BF16, tag="bd0")
        bd1 = bsb.tile([64, C], BF16, tag="bd1")
        nc.gpsimd.memset(bd0[:], 0.0)
        nc.gpsimd.memset(bd1[:], 0.0)
        nc.sync.dma_start(bd0[0:48, 0:48], attn_bf[:, 0:48])
        nc.sync.dma_start(bd0[48:96, 48:96], attn_bf[:, 48:96])
        nc.sync.dma_start(bd0[96:128, 96:144], attn_bf[0:32, 96:144])
        nc.sync.dma_start(bd1[0:16, 96:144], attn_bf[32:48, 96:144])
        nc.sync.dma_start(bd1[16:64, 144:192], attn_bf[:, 144:192])

        # W_effT = BD(attn).T @ W_outT   [192 x 192], bf16
        weff0 = bsb.tile([128, 256], BF16, tag="weff0")
        weff1 = bsb.tile([128, 256], BF16, tag="weff1")
        nc.gpsimd.memset(weff0[:], 0.0)
        nc.gpsimd.memset(weff1[:], 0.0)
        for m0, m1, wt in [(0, 128, weff0), (128, 192, weff1)]:
            pw = bps.tile([128, C], F32, tag="pweff")
            nc.tensor.matmul(pw[0 : m1 - m0, :], bd0[:, m0:m1], woutt0_bf[:], start=True, stop=False)
            nc.tensor.matmul(pw[0 : m1 - m0, :], bd1[:, m0:m1], woutt1_bf[:], start=False, stop=True)
            copy(wt[0 : m1 - m0, 0:C], pw[0 : m1 - m0, :])

        # y = W_effT.T @ v
        for g in range(L // 512):
            sl = slice(g * 512, (g + 1) * 512)
            for m0, m1 in [(0, 128), (128, 192)]:
                po = ops.tile([128, 512], F32, tag="ops")
                nc.tensor.matmul(po[:, :], weff0[:, m0 : m0 + 128], v0[:, sl], start=True, stop=False)
                nc.tensor.matmul(po[:, :], weff1[:, m0 : m0 + 128], v1[:, sl], start=False, stop=True)
                ot = osb.tile([m1 - m0, 512], F32, tag=f"o{m0}", name=f"o{m0}")
                copy(ot[:], po[0 : m1 - m0, :])
                nc.sync.dma_start(y_d[m0:m1, sl], ot[:])


def _tap_idx(di, dj):
    return 3 * (di + 1) + (dj + 1)


def _prep_weights(w_proj1, w_dw, w_out):
    import ml_dtypes

    E4 = ml_dtypes.float8_e4m3
    w1t = np.asarray(w_proj1, np.float32).reshape(CQKV, C).T  # [in, out]
    wdw = np.asarray(w_dw, np.float32).reshape(CQKV, 9)
    woutt = np.ascontiguousarray(np.asarray(w_out, np.float32).reshape(C, C).T)

    w1qk = np.zeros((128, 2, 384), np.float32)
    w1qk[:, 0, :] = w1t[0:128, 0:384] * S1
    w1qk[0:64, 1, :] = w1t[128:192, 0:384] * S1
    w1qk = np.ascontiguousarray(w1qk.reshape(128, 768)).astype(E4)

    w1tv = np.ascontiguousarray(w1t[:, 384:576]).astype(ml_dtypes.bfloat16)

    wqk8 = np.zeros((128, 30, 128), np.float32)
    for b in range(3):
        for p in range(5):
            for s in range(2):
                t = 2 * p + s
                if t < 9:
                    np.fill_diagonal(wqk8[:, (b * 5 + p) * 2 + s, :], wdw[b * 128 : (b + 1) * 128, t] * S2)
    wqk8 = np.ascontiguousarray(wqk8.reshape(128, 3840)).astype(E4)

    w3 = np.zeros((128, 9, 128), np.float32)
    for t in range(9):
        np.fill_diagonal(w3[:, t, :], wdw[384:512, t])
    w3 = np.ascontiguousarray(w3.reshape(128, 1152)).astype(ml_dtypes.bfloat16)

    w4 = np.zeros((128, 6, 64), np.float32)
    for i, di in enumerate((-1, 0, 1)):
        np.fill_diagonal(w4[0:64, 2 * i, :], wdw[512:576, _tap_idx(di, -1)])
        np.fill_diagonal(w4[64:128, 2 * i, :], wdw[512:576, _tap_idx(di, 0)])
        np.fill_diagonal(w4[0:64, 2 * i + 1, :], wdw[512:576, _tap_idx(di, 1)])
    w4 = np.ascontiguousarray(w4.reshape(128, 384)).astype(ml_dtypes.bfloat16)

    w4s = np.zeros((64, 9, 64), np.float32)
    for t in range(9):
        np.fill_diagonal(w4s[:, t, :], wdw[512:576, t])
    w4s = np.ascontiguousarray(w4s.reshape(64, 576)).astype(ml_dtypes.bfloat16)

    wqs = np.zeros((128, 27, 128), np.float32)
    for b in range(3):
        for t in range(9):
            np.fill_diagonal(wqs[:, b * 9 + t, :], wdw[b * 128 : (b + 1) * 128, t])
    wqs = np.ascontiguousarray(wqs.reshape(128, 27 * 128)).astype(ml_dtypes.bfloat16)

    return {
        "w1qk": w1qk,
        "w1tv": w1tv,
        "wqk8": wqk8,
        "w3": w3,
        "w4": w4,
        "w4s": w4s,
        "wqs": wqs,
        "woutt": woutt,
    }


_NC_CACHE = None


def _get_nc():
    global _NC_CACHE
    if _NC_CACHE is None:
        _NC_CACHE = build_nc()
    return _NC_CACHE


def kernel(x, w_proj1, w_dw, pos_emb, w_out, _trace=False):
    from concourse.bass_utils import run_bass_kernel_spmd

    import ml_dtypes

    xf = np.asarray(x, dtype=np.float32)
    xbf = xf.astype(ml_dtypes.bfloat16)
    xf8 = xf.astype(ml_dtypes.float8_e4m3)
    wmaps = _prep_weights(w_proj1, w_dw, w_out)
    # pos_emb adds a per-head constant to every logit in its softmax row;
    # softmax is shift-invariant, so it has no effect on the output.

    nc = _get_nc()
    in_maps = [
        {
            "x": np.ascontiguousarray(xbf[b].reshape(C, L)),
            "xf8": np.ascontiguousarray(xf8[b].reshape(C, L)),
            **wmaps,
        }
        for b in range(N_CORES)
    ]
    res = run_bass_kernel_spmd(nc, in_maps, list(range(N_CORES)), trace=_trace)
    out = np.stack([res.results[b]["y"].reshape(C, H, W) for b in range(N_CORES)])
    if _trace:
        kernel.last_exec_time_ns = res.exec_time_ns
        kernel.last_profile = res
    return out.astype(np.float32)


# revision 15
# speedup vs baseline: 1.0233x; 1.0000x over previous
"""ChannelSA Trainium2 kernel: 8-way batch-parallel across NeuronCores.

kernel(**inputs) takes the FULL inputs (x [8,192,128,128], conv weights,
pos_emb) and returns the FULL output [8,192,128,128] fp32. Each core runs
an identical single-batch program (SPMD, no collectives).

Per-core pipeline (v2: fp8 DoubleRow on the q,k path):
  q,k path is scale-invariant (logits are the normalized Gram
  Gqk/(|q||k|), softmax is shift-invariant) so it runs entirely in
  fp8e4m3 with prescaled weights at the PE's DoubleRow rate (2 K-tiles
  per pass, 0.5 cyc/col):
    zqk = W1qk @ x        one DR matmul per 128-out block (K=192 in 2 k-tiles)
    q,k = DW3x3(zqk)      9 taps as 5 DR diag-pair matmuls per block
  v path stays bf16 for accuracy:
    zv  = W1v @ x         K=128+64 accumulating matmuls
    v   = DW3x3(zv)       block3: 9 diag matmuls; block4 (64ch): z stored
                          twice (partitions 64:128 col-shifted) so taps
                          (di,-1)+(di,0) share one matmul -> 6 matmuls
  q,k -> bf16 DMA-transpose -> per-head Gram banks [Gqk|Gqq|Gkk] in PSUM
  logits = Gqk / (|q||k| sqrt(L))  (pos_emb is a per-row constant: no-op)
  attn = softmax(logits)
  y = (W_out @ blockdiag(attn)) @ v
"""
import math
from contextlib import ExitStack

import numpy as np

import concourse.ap as cap
import concourse.bass as bass
import concourse.mybir as mybir
import concourse.tile as tile
from concourse.masks import make_identity

F32 = mybir.dt.float32
BF16 = mybir.dt.bfloat16
F8 = mybir.dt.float8e4
AF = mybir.ActivationFunctionType
DR = mybir.MatmulPerfMode.DoubleRow

C = 192
CQKV = 576
H = 128
W = 128
L = H * W
HEADS = 4
DH = 48
R = 8                    # output image rows per chunk
NCHUNK = H // R
PADW = W + 2             # padded row stride in z tiles
ZROWS = R + 2            # rows held per z chunk (1 halo each side)
TAPS = [(di, dj) for di in (-1, 0, 1) for dj in (-1, 0, 1)]
N_CORES = 8
BLK4_PAIR = True        # block4 dw tap-pair trick (partition dup)
FP8_DW = 2               # q,k dw mode: 0=bf16 diag, 1=fp8 DoubleRow pairs, 2=fp8 singles
SPLIT_C1 = False         # conv1-qk as two plain fp8 matmuls instead of one DR
S1 = 16.0                # fp8 prescale on W1 qk columns
S2 = 4.0                 # fp8 prescale on dw qk weights

_MAX_DRAIN_WAITS = 1


def _patch_tail_drain():
    """The walrus in this image rejects >1 semaphore wait on the Tile tail
    drain instruction; split the waits across a chain of SP nops."""
    if getattr(tile.TileContext, "_drain_patched", False):
        return

    def _drain_and_barrier(self, tick_clock, wait_clock):
        from concourse.vector_clock import ScopedClock

        nc = self.nc
        drain_inst = nc.sync.drain()
        wait_clock.add_sem_waits(
            drain_inst.ins, ScopedClock({None: tick_clock.global_clock})
        )
        si = drain_inst.ins.sync_info
        waits = list(si.on_wait or [])
        if len(waits) > _MAX_DRAIN_WAITS:
            si.on_wait = waits[:_MAX_DRAIN_WAITS]
            rest = waits[_MAX_DRAIN_WAITS:]
            for i in range(0, len(rest), _MAX_DRAIN_WAITS):
                nop = nc.sync.nop(nofuse=True)
                nop.ins.sync_info = mybir.SyncInfo(
                    on_wait=rest[i : i + _MAX_DRAIN_WAITS], on_update=[]
                )
        nc.all_engine_barrier()
        assert self.sems is not None
        popped = nc._tile_sem_poison_stack.pop()
        assert popped is self._sem_poison
        nc.clear_and_free_semaphores(list(self.sems.allocated().values()))
        nc.all_engine_barrier()

    tile.TileContext._drain_and_barrier = _drain_and_barrier
    tile.TileContext._drain_patched = True


def build_nc(split_waits=True):
    _patch_tail_drain()
    nc = bass.Bass("TRN2", target_bir_lowering=False, debug=False)

    x_d = nc.declare_dram_parameter("x", [C, L], BF16, isOutput=False)
    xf8_d = nc.declare_dram_parameter("xf8", [C, L], F8, isOutput=False)
    w1qk_d = nc.declare_dram_parameter("w1qk", [128, 2 * 384], F8, isOutput=False)
    w1tv_d = nc.declare_dram_parameter("w1tv", [C, C], BF16, isOutput=False)
    wqk8_d = nc.declare_dram_parameter("wqk8", [128, 30 * 128], F8, isOutput=False)
    w3_d = nc.declare_dram_parameter("w3", [128, 9 * 128], BF16, isOutput=False)
    w4_d = nc.declare_dram_parameter("w4", [128, 6 * 64], BF16, isOutput=False)
    w4s_d = nc.declare_dram_parameter("w4s", [64, 9 * 64], BF16, isOutput=False)
    wqs_d = nc.declare_dram_parameter("wqs", [128, 27 * 128], BF16, isOutput=False)
    woutt_d = nc.declare_dram_parameter("woutt", [C, C], F32, isOutput=False)
    y_d = nc.declare_dram_parameter("y", [C, L], F32, isOutput=True)

    with tile.TileContext(nc) as tc, ExitStack() as ctx:
        _body(ctx, tc, x_d, xf8_d, w1qk_d, w1tv_d, wqk8_d, w3_d, w4_d, w4s_d, wqs_d, woutt_d, y_d)
    if split_waits:
        _split_excess_waits(nc)
    return nc


def _split_excess_waits(nc, maxw=1):
    """This walrus build accepts only one semaphore wait per instruction.
    Move excess waits onto same-engine no-ops inserted just before the
    offending instruction (same-engine program order preserves semantics)."""
    uid = [0]
    for f in nc.m.functions:
        for bb in f.blocks:
            il = bb.instructions
            out = []
            changed = False
            for inst in il:
                si = inst.sync_info
                waits = list(si.on_wait) if si and si.on_wait else []
                if len(waits) > maxw:
                    changed = True
                    rest, keep = waits[:-maxw], waits[-maxw:]
                    for i in range(0, len(rest), maxw):
                        uid[0] += 1
                        out.append(
                            mybir.InstNoOp(
                                name=f"I-waitsplit-{uid[0]}",
                                engine=inst.engine,
                                ins=[],
                                outs=[],
                                sync_info=mybir.SyncInfo(
                                    on_wait=rest[i : i + maxw], on_update=[]
                                ),
                            )
                        )
                    si.on_wait = keep
                out.append(inst)
            if changed:
                bb.instructions = out


def _tap_pair_ap(ztile, orow, p):
    """rhs AP [128, 2(k-tiles), 4(rows), W] for dw tap pair p over a
    [128, ZROWS, PADW] z tile; k-tile t reads tap TAPS[2p+t]."""
    t0 = TAPS[2 * p]
    off0 = (orow + t0[0]) * PADW + (1 + t0[1])
    if 2 * p + 1 < 9:
        t1 = TAPS[2 * p + 1]
        delta = (orow + t1[0]) * PADW + (1 + t1[1]) - off0
    else:
        delta = 0  # pad pair: zero weights, reread tap t0 (stride-0 is legal)
    base = ztile[:]
    pstride = base.ap[0][0]
    return cap.AP(
        base.tensor,
        base.offset + off0,
        [[pstride, 128], [delta, 2], [PADW, 4], [1, W]],
    )


def _body(ctx, tc, x_d, xf8_d, w1qk_d, w1tv_d, wqk8_d, w3_d, w4_d, w4s_d, wqs_d, woutt_d, y_d):
    nc = tc.nc
    ncopy = [0]

    def copy(dst, src):
        # alternate PSUM->SBUF copies between ACT and DVE
        if ncopy[0] % 2 == 0:
            nc.scalar.copy(dst, src)
        else:
            nc.vector.tensor_copy(dst, src)
        ncopy[0] += 1

    const = ctx.enter_context(tc.tile_pool(name="const", bufs=1))
    persist = ctx.enter_context(tc.tile_pool(name="persist", bufs=1))

    # ---- constants / weights ----
    w1qk = const.tile([128, 2, 384], F8, tag="w1qk")
    nc.sync.dma_start(w1qk[:], w1qk_d[:].rearrange("k (t m) -> k t m", t=2))
    wqk8 = const.tile([128, 30, 128], F8, tag="wqk8")
    nc.sync.dma_start(wqk8[:], wqk8_d[:].rearrange("k (p m) -> k p m", p=30))
    w3 = const.tile([128, 9, 128], BF16, tag="w3")
    nc.sync.dma_start(w3[:], w3_d[:].rearrange("k (t m) -> k t m", t=9))
    w4 = const.tile([128, 6, 64], BF16, tag="w4")
    nc.sync.dma_start(w4[:], w4_d[:].rearrange("k (t m) -> k t m", t=6))
    w4s = const.tile([64, 9, 64], BF16, tag="w4s")
    nc.sync.dma_start(w4s[:], w4s_d[:].rearrange("k (t m) -> k t m", t=9))
    if not FP8_DW:
        wqs = const.tile([128, 27, 128], BF16, tag="wqs")
        nc.sync.dma_start(wqs[:], wqs_d[:].rearrange("k (t m) -> k t m", t=27))

    w1tv0 = const.tile([128, C], BF16, tag="w1tv0")
    w1tv1 = const.tile([128, C], BF16, tag="w1tv1")
    nc.gpsimd.memset(w1tv1[:], 0.0)
    nc.sync.dma_start(w1tv0[:], w1tv_d[0:128, :])
    nc.sync.dma_start(w1tv1[0:64, :], w1tv_d[128:192, :])

    woutt0 = const.tile([128, C], F32, tag="woutt0")
    woutt1 = const.tile([64, C], F32, tag="woutt1")
    nc.sync.dma_start(woutt0[:], woutt_d[0:128, :])
    nc.sync.dma_start(woutt1[:], woutt_d[128:192, :])
    woutt0_bf = const.tile([128, C], BF16, tag="woutt0bf")
    woutt1_bf = const.tile([64, C], BF16, tag="woutt1bf")
    nc.vector.tensor_copy(woutt0_bf[:], woutt0[:])
    nc.vector.tensor_copy(woutt1_bf[:], woutt1[:])

    ident48 = const.tile([48, 48], F32, tag="ident48")
    make_identity(nc, ident48[:])
    ones48 = const.tile([48, 1], F32, tag="ones48")
    nc.gpsimd.memset(ones48[:], 1.0)
    ones1x48 = const.tile([1, 48], F32, tag="ones1x48")
    nc.gpsimd.memset(ones1x48[:], 1.0)

    # ---- persistent state ----
    v0 = persist.tile([128, L], BF16, tag="v0")
    v1 = persist.tile([128, L], BF16, tag="v1")
    nc.gpsimd.memset(v1[64:128, :], 0.0)
    ZQDT = F8 if FP8_DW else BF16
    zqk = [
        [
            persist.tile([128, ZROWS, PADW], ZQDT, tag=f"zqk{s}_{b}", name=f"zqk{s}_{b}")
            for b in range(3)
        ]
        for s in range(2)
    ]
    zv3 = [persist.tile([128, ZROWS, PADW], BF16, tag=f"zv3_{s}", name=f"zv3_{s}") for s in range(2)]
    zv4 = [persist.tile([128, ZROWS, PADW], BF16, tag=f"zv4_{s}", name=f"zv4_{s}") for s in range(2)]
    for s in range(2):
        for b in range(3):
            nc.gpsimd.memset(zqk[s][b][:], 0.0)
        nc.gpsimd.memset(zv3[s][:], 0.0)
        nc.gpsimd.memset(zv4[s][:], 0.0)

    ghs = persist.tile([48, HEADS * 144], F32, tag="ghs")
    xt1_pp = [persist.tile([128, ZROWS, W], BF16, tag=f"xt1_{s}", name=f"xt1_{s}") for s in range(2)]
    xf8_pp = [
        persist.tile([128, 2, ZROWS, W], F8, tag=f"xf8_{s}", name=f"xf8_{s}") for s in range(2)
    ]
    for s in range(2):
        nc.gpsimd.memset(xt1_pp[s][:], 0.0)
        nc.gpsimd.memset(xf8_pp[s][:], 0.0)

    # ---- phase A: chunked pipeline ----
    with (
        tc.tile_pool(name="gps", bufs=1, space="PSUM") as gps,
        tc.tile_pool(name="xp", bufs=2) as xp,
        tc.tile_pool(name="zps", bufs=3, space="PSUM") as zps,
        tc.tile_pool(name="qps", bufs=2, space="PSUM") as qps,
        tc.tile_pool(name="stp", bufs=2) as stp,
        tc.tile_pool(name="qktp", bufs=2) as qktp,
    ):
        # two G banks; a single accumulation group spans all heads per bank
        g1 = gps.tile([48, HEADS * 96], F32, tag="g1")
        g2 = gps.tile([48, HEADS * 48], F32, tag="g2")
        for c in range(NCHUNK):
            zq = zqk[c % 2]
            z3 = zv3[c % 2]
            z4 = zv4[c % 2]
            r0 = max(0, R * c - 1)
            r1 = min(H, R * c + R + 1)
            nrows = r1 - r0
            brow0 = r0 - (R * c - 1)  # buf row of image row r0

            xt0 = xp.tile([128, nrows, W], BF16, tag="x0")
            xt1 = xt1_pp[c % 2]
            xf8 = xf8_pp[c % 2]
            nc.sync.dma_start(
                xt0[:], x_d[0:128, r0 * W : r1 * W].rearrange("p (r w) -> p r w", w=W)
            )
            nc.sync.dma_start(
                xt1[0:64, 0:nrows, :],
                x_d[128:192, r0 * W : r1 * W].rearrange("p (r w) -> p r w", w=W),
            )
            nc.sync.dma_start(
                xf8[:, 0, 0:nrows, :],
                xf8_d[0:128, r0 * W : r1 * W].rearrange("p (r w) -> p r w", w=W),
            )
            nc.sync.dma_start(
                xf8[0:64, 1, 0:nrows, :],
                xf8_d[128:192, r0 * W : r1 * W].rearrange("p (r w) -> p r w", w=W),
            )

            # conv1 into padded z tiles (groups of <=4 rows)
            for g0 in range(0, nrows, 4):
                gn = min(4, nrows - g0)
                rsl = slice(brow0 + g0, brow0 + g0 + gn)
                for b in range(3):
                    ps = zps.tile([128, 512], F32, tag="zps")
                    if SPLIT_C1:
                        nc.tensor.matmul(
                            ps[:, 0 : gn * W],
                            w1qk[:, 0, b * 128 : (b + 1) * 128],
                            xf8[:, 0, g0 : g0 + gn, :],
                            start=True,
                            stop=False,
                        )
                        nc.tensor.matmul(
                            ps[:, 0 : gn * W],
                            w1qk[:, 1, b * 128 : (b + 1) * 128],
                            xf8[:, 1, g0 : g0 + gn, :],
                            start=False,
                            stop=True,
                        )
                    else:
                        nc.tensor.matmul(
                            ps[:, 0 : gn * W],
                            w1qk[:, :, b * 128 : (b + 1) * 128],
                            xf8[:, :, g0 : g0 + gn, :],
                            start=True,
                            stop=True,
                            perf_mode=DR,
                        )
                    copy(zq[b][:, rsl, 1 : 1 + W], ps[:, 0 : gn * W])
                ps3 = zps.tile([128, 512], F32, tag="zps")
                nc.tensor.matmul(
                    ps3[:, 0 : gn * W], w1tv0[:, 0:128], xt0[:, g0 : g0 + gn, :],
                    start=True, stop=False,
                )
                nc.tensor.matmul(
                    ps3[:, 0 : gn * W], w1tv1[:, 0:128], xt1[:, g0 : g0 + gn, :],
                    start=False, stop=True,
                )
                copy(z3[:, rsl, 1 : 1 + W], ps3[:, 0 : gn * W])
                ps4 = zps.tile([128, 512], F32, tag="zps")
                nc.tensor.matmul(
                    ps4[0:64, 0 : gn * W], w1tv0[:, 128:192], xt0[:, g0 : g0 + gn, :],
                    start=True, stop=False,
                )
                nc.tensor.matmul(
                    ps4[0:64, 0 : gn * W], w1tv1[:, 128:192], xt1[:, g0 : g0 + gn, :],
                    start=False, stop=True,
                )
                copy(z4[0:64, rsl, 1 : 1 + W], ps4[0:64, 0 : gn * W])
                if BLK4_PAIR:
                    # partition-shifted duplicate (col -1) for the dw tap-pair
                    # trick; engines can't cross partitions, so DMA the bf16 copy
                    nc.scalar.dma_start(z4[64:128, rsl, 0:W], z4[0:64, rsl, 1 : 1 + W])

            if c == NCHUNK - 1:
                # bottom halo row never written this chunk; clear stale data
                for b in range(3):
                    nc.gpsimd.memset(zq[b][:, ZROWS - 1 : ZROWS, :], 0.0)
                nc.gpsimd.memset(z3[:, ZROWS - 1 : ZROWS, :], 0.0)
                nc.gpsimd.memset(z4[:, ZROWS - 1 : ZROWS, :], 0.0)

            # dw taps -> qkv rows Rc..Rc+R
            st = [stp.tile([128, R // 4, 4 * W], BF16, tag=f"st{i}", name=f"st{i}") for i in range(3)]
            for g in range(R // 4):
                orow = 1 + 4 * g  # buf row of first output row in this group
                for b in range(3):
                    ps = qps.tile([128, 512], F32, tag="qps")
                    if FP8_DW == 1:
                        for p in range(5):
                            nc.tensor.matmul(
                                ps[:, :],
                                wqk8[:, (b * 5 + p) * 2 : (b * 5 + p) * 2 + 2, :],
                                _tap_pair_ap(zq[b], orow, p),
                                start=(p == 0),
                                stop=(p == 4),
                                perf_mode=DR,
                            )
                    elif FP8_DW == 2:
                        for t, (di, dj) in enumerate(TAPS):
                            nc.tensor.matmul(
                                ps[:, :],
                                wqk8[:, (b * 5 + t // 2) * 2 + t % 2, :],
                                zq[b][:, orow + di : orow + di + 4, 1 + dj : 1 + dj + W],
                                start=(t == 0),
                                stop=(t == 8),
                            )
                    else:
                        for t, (di, dj) in enumerate(TAPS):
                            nc.tensor.matmul(
                                ps[:, :],
                                wqs[:, b * 9 + t, :],
                                zq[b][:, orow + di : orow + di + 4, 1 + dj : 1 + dj + W],
                                start=(t == 0),
                                stop=(t == 8),
                            )
                    copy(st[b][:, g, :], ps[:, :])
                ps3 = qps.tile([128, 512], F32, tag="qps")
                for t, (di, dj) in enumerate(TAPS):
                    nc.tensor.matmul(
                        ps3[:, :],
                        w3[:, t, :],
                        z3[:, orow + di : orow + di + 4, 1 + dj : 1 + dj + W],
                        start=(t == 0),
                        stop=(t == 8),
                    )
                copy(v0[:, c * R * W + g * 512 : c * R * W + (g + 1) * 512], ps3[:, :])
                ps4 = qps.tile([64, 512], F32, tag="qps4", bufs=1)
                if BLK4_PAIR:
                    for i, di in enumerate((-1, 0, 1)):
                        nc.tensor.matmul(
                            ps4[:, :],
                            w4[:, 2 * i, :],
                            z4[:, orow + di : orow + di + 4, 0:W],
                            start=(i == 0),
                            stop=False,
                        )
                        nc.tensor.matmul(
                            ps4[:, :],
                            w4[0:64, 2 * i + 1, :],
                            z4[0:64, orow + di : orow + di + 4, 2 : 2 + W],
                            start=False,
                            stop=(i == 2),
                        )
                else:
                    for t, (di, dj) in enumerate(TAPS):
                        nc.tensor.matmul(
                            ps4[:, :],
                            w4s[0:64, t, :],
                            z4[0:64, orow + di : orow + di + 4, 1 + dj : 1 + dj + W],
                            start=(t == 0),
                            stop=(t == 8),
                        )
                copy(v1[0:64, c * R * W + g * 512 : c * R * W + (g + 1) * 512], ps4[:, :])

            # transpose q,k: qkt[:, lt, 0, :] = k^T, [:, lt, 1, :] = q^T
            st_flat = [s.rearrange("p a b -> p (a b)") for s in st]
            qkt = qktp.tile([128, R, 2, 192], BF16, tag="qkt")
            nc.sync.dma_start_transpose(qkt[:, :, 1, 0:128], st_flat[0][:, :])
            nc.scalar.dma_start_transpose(qkt[:, :, 1, 128:192], st_flat[1][0:64, :])
            nc.sync.dma_start_transpose(qkt[:, :, 0, 0:64], st_flat[1][64:128, :])
            nc.scalar.dma_start_transpose(qkt[:, :, 0, 64:192], st_flat[2][:, :])

            # gram accumulation
            for lt in range(R):
                first = c == 0 and lt == 0
                last = c == NCHUNK - 1 and lt == R - 1
                for h in range(HEADS):
                    nc.tensor.matmul(
                        g1[:, h * 96 : h * 96 + 96],
                        qkt[:, lt, 1, h * DH : (h + 1) * DH],
                        qkt[:, lt, :, h * DH : (h + 1) * DH],
                        start=(first and h == 0),
                        stop=(last and h == HEADS - 1),
                        skip_group_check=True,
                    )
                    nc.tensor.matmul(
                        g2[:, h * DH : (h + 1) * DH],
                        qkt[:, lt, 0, h * DH : (h + 1) * DH],
                        qkt[:, lt, 0, h * DH : (h + 1) * DH],
                        start=(first and h == 0),
                        stop=(last and h == HEADS - 1),
                        skip_group_check=True,
                    )

        nc.vector.tensor_copy(ghs[:, 0 : HEADS * 96], g1[:])
        nc.vector.tensor_copy(ghs[:, HEADS * 96 :], g2[:])

    # ---- phase B ----
    with (
        tc.tile_pool(name="bsb", bufs=1) as bsb,
        tc.tile_pool(name="bps", bufs=1, space="PSUM") as bps,
        tc.tile_pool(name="ops", bufs=4, space="PSUM") as ops,
        tc.tile_pool(name="osb", bufs=4) as osb,
    ):
        attn_bf = bsb.tile([48, HEADS * 48], BF16, tag="attnbf")
        scr = bsb.tile([48, 48], F32, tag="scr")
        scr2 = bsb.tile([48, 48], F32, tag="scr2")
        colv = bsb.tile([48, 1], F32, tag="colv")
        rowv = bsb.tile([1, 48], F32, tag="rowv")
        rkrep = bsb.tile([48, 48], F32, tag="rkrep")
        logits = bsb.tile([48, 48], F32, tag="logits")

        for h in range(HEADS):
            gqk = ghs[:, h * 96 : h * 96 + 48]
            gqq = ghs[:, h * 96 + 48 : h * 96 + 96]
            gkk = ghs[:, HEADS * 96 + h * DH : HEADS * 96 + (h + 1) * DH]

            # rq_inv = 1/max(sqrt(diag(Gqq)),eps); 1/sqrt(L) scale is constant
            # across the softmax row only if folded for both q and k norms
            nc.vector.tensor_mul(scr[:], gqq, ident48[:])
            nc.vector.reduce_sum(colv[:], scr[:], axis=mybir.AxisListType.X)
            nc.scalar.activation(colv[:], colv[:], AF.Sqrt)
            nc.vector.tensor_scalar_max(colv[:], colv[:], 1e-12)
            nc.vector.reciprocal(colv[:], colv[:])
            nc.vector.tensor_scalar(
                logits[:],
                gqk,
                colv[:],
                1.0 / math.sqrt(L),
                op0=mybir.AluOpType.mult,
                op1=mybir.AluOpType.mult,
            )

            # rk_inv broadcast along the free (key) dim via diag-as-row
            nc.vector.tensor_mul(scr2[:], gkk, ident48[:])
            ps_row = bps.tile([1, 48], F32, tag="pssmall")
            nc.tensor.matmul(ps_row[:], ones48[:], scr2[:], start=True, stop=True)
            nc.vector.tensor_copy(rowv[:], ps_row[:])
            nc.scalar.activation(rowv[:], rowv[:], AF.Sqrt)
            nc.vector.tensor_scalar_max(rowv[:], rowv[:], 1e-12)
            nc.vector.reciprocal(rowv[:], rowv[:])
            ps_rep = bps.tile([48, 48], F32, tag="pssmall")
            nc.tensor.matmul(ps_rep[:], ones1x48[:], rowv[:], start=True, stop=True)
            nc.vector.tensor_copy(rkrep[:], ps_rep[:])
            nc.vector.tensor_mul(logits[:], logits[:], rkrep[:])

            # softmax over the free (key) dim
            nc.vector.reduce_max(colv[:], logits[:], axis=mybir.AxisListType.X)
            nc.vector.tensor_scalar_sub(logits[:], logits[:], colv[:])
            nc.scalar.activation(logits[:], logits[:], AF.Exp)
            nc.vector.reduce_sum(colv[:], logits[:], axis=mybir.AxisListType.X)
            nc.vector.reciprocal(colv[:], colv[:])
            nc.vector.tensor_scalar_mul(logits[:], logits[:], colv[:])
            nc.vector.tensor_copy(attn_bf[:, h * 48 : (h + 1) * 48], logits[:])

        # block-diagonal attn (bf16)
        bd0 = bsb.tile([128, C], BF16, tag="bd0")
        bd1 = bsb.tile([64, C], BF16, tag="bd1")
        nc.gpsimd.memset(bd0[:], 0.0)
        nc.gpsimd.memset(bd1[:], 0.0)
        nc.sync.dma_start(bd0[0:48, 0:48], attn_bf[:, 0:48])
        nc.sync.dma_start(bd0[48:96, 48:96], attn_bf[:, 48:96])
        nc.sync.dma_start(bd0[96:128, 96:144], attn_bf[0:32, 96:144])
        nc.sync.dma_start(bd1[0:16, 96:144], attn_bf[32:48, 96:144])
        nc.sync.dma_start(bd1[16:64, 144:192], attn_bf[:, 144:192])

        # W_effT = BD(attn).T @ W_outT   [192 x 192], bf16
        weff0 = bsb.tile([128, 256], BF16, tag="weff0")
        weff1 = bsb.tile([128, 256], BF16, tag="weff1")
        nc.gpsimd.memset(weff0[:], 0.0)
        nc.gpsimd.memset(weff1[:], 0.0)
        for m0, m1, wt in [(0, 128, weff0), (128, 192, weff1)]:
            pw = bps.tile([128, C], F32, tag="pweff")
            nc.tensor.matmul(pw[0 : m1 - m0, :], bd0[:, m0:m1], woutt0_bf[:], start=True, stop=False)
            nc.tensor.matmul(pw[0 : m1 - m0, :], bd1[:, m0:m1], woutt1_bf[:], start=False, stop=True)
            copy(wt[0 : m1 - m0, 0:C], pw[0 : m1 - m0, :])

        # y = W_effT.T @ v
        for g in range(L // 512):
            sl = slice(g * 512, (g + 1) * 512)
            for m0, m1 in [(0, 128), (128, 192)]:
                po = ops.tile([128, 512], F32, tag="ops")
                nc.tensor.matmul(po[:, :], weff0[:, m0 : m0 + 128], v0[:, sl], start=True, stop=False)
                nc.tensor.matmul(po[:, :], weff1[:, m0 : m0 + 128], v1[:, sl], start=False, stop=True)
                ot = osb.tile([m1 - m0, 512], F32, tag=f"o{m0}", name=f"o{m0}")
                copy(ot[:], po[0 : m1 - m0, :])
                nc.sync.dma_start(y_d[m0:m1, sl], ot[:])


def _tap_idx(di, dj):
    return 3 * (di + 1) + (dj + 1)


def _prep_weights(w_proj1, w_dw, w_out):
    import ml_dtypes

    E4 = ml_dtypes.float8_e4m3
    w1t = np.asarray(w_proj1, np.float32).reshape(CQKV, C).T  # [in, out]
    wdw = np.asarray(w_dw, np.float32).reshape(CQKV, 9)
    woutt = np.ascontiguousarray(np.asarray(w_out, np.float32).reshape(C, C).T)

    w1qk = np.zeros((128, 2, 384), np.float32)
    w1qk[:, 0, :] = w1t[0:128, 0:384] * S1
    w1qk[0:64, 1, :] = w1t[128:192, 0:384] * S1
    w1qk = np.ascontiguousarray(w1qk.reshape(128, 768)).astype(E4)

    w1tv = np.ascontiguousarray(w1t[:, 384:576]).astype(ml_dtypes.bfloat16)

    wqk8 = np.zeros((128, 30, 128), np.float32)
    for b in range(3):
        for p in range(5):
            for s in range(2):
                t = 2 * p + s
                if t < 9:
                    np.fill_diagonal(wqk8[:, (b * 5 + p) * 2 + s, :], wdw[b * 128 : (b + 1) * 128, t] * S2)
    wqk8 = np.ascontiguousarray(wqk8.reshape(128, 3840)).astype(E4)

    w3 = np.zeros((128, 9, 128), np.float32)
    for t in range(9):
        np.fill_diagonal(w3[:, t, :], wdw[384:512, t])
    w3 = np.ascontiguousarray(w3.reshape(128, 1152)).astype(ml_dtypes.bfloat16)

    w4 = np.zeros((128, 6, 64), np.float32)
    for i, di in enumerate((-1, 0, 1)):
        np.fill_diagonal(w4[0:64, 2 * i, :], wdw[512:576, _tap_idx(di, -1)])
        np.fill_diagonal(w4[64:128, 2 * i, :], wdw[512:576, _tap_idx(di, 0)])
        np.fill_diagonal(w4[0:64, 2 * i + 1, :], wdw[512:576, _tap_idx(di, 1)])
    w4 = np.ascontiguousarray(w4.reshape(128, 384)).astype(ml_dtypes.bfloat16)

    w4s = np.zeros((64, 9, 64), np.float32)
    for t in range(9):
        np.fill_diagonal(w4s[:, t, :], wdw[512:576, t])
    w4s = np.ascontiguousarray(w4s.reshape(64, 576)).astype(ml_dtypes.bfloat16)

    wqs = np.zeros((128, 27, 128), np.float32)
    for b in range(3):
        for t in range(9):
            np.fill_diagonal(wqs[:, b * 9 + t, :], wdw[b * 128 : (b + 1) * 128, t])
    wqs = np.ascontiguousarray(wqs.reshape(128, 27 * 128)).astype(ml_dtypes.bfloat16)

    return {
        "w1qk": w1qk,
        "w1tv": w1tv,
        "wqk8": wqk8,
        "w3": w3,
        "w4": w4,
        "w4s": w4s,
        "wqs": wqs,
        "woutt": woutt,
    }


_NC_CACHE = None


def _get_nc():
    global _NC_CACHE
    if _NC_CACHE is None:
        _NC_CACHE = build_nc()
    return _NC_CACHE


def kernel(x, w_proj1, w_dw, pos_emb, w_out, _trace=False):
    from concourse.bass_utils import run_bass_kernel_spmd

    import ml_dtypes

    xf = np.asarray(x, dtype=np.float32)
    xbf = xf.astype(ml_dtypes.bfloat16)
    xf8 = xf.astype(ml_dtypes.float8_e4m3)
    wmaps = _prep_weights(w_proj1, w_dw, w_out)
    # pos_emb adds a per-head constant to every logit in its softmax row;
    # softmax is shift-invariant, so it has no effect on the output.

    nc = _get_nc()
    in_maps = [
        {
            "x": np.ascontiguousarray(xbf[b].reshape(C, L)),
            "xf8": np.ascontiguousarray(xf8[b].reshape(C, L)),
            **wmaps,
        }
        for b in range(N_CORES)
    ]
    res = run_bass_kernel_spmd(nc, in_maps, list(range(N_CORES)), trace=_trace)
    out = np.stack([res.results[b]["y"].reshape(C, H, W) for b in range(N_CORES)])
    if _trace:
        kernel.last_exec_time_ns = res.exec_time_ns
        kernel.last_profile = res
    return out.astype(np.float32)
